# revision 1
# baseline (speedup 1.0000x reference)
"""Trainium2 Bass kernel for nn_MultiHeadAttention (B=2, S=2048, D=1024, H=16).

Sharding: 8 cores = 2 (batch) x 4 (head groups of 4 heads / 256 dims).
Each core computes QKV projections for its head slice, attention for its 4
heads, and the partial output projection for its 256-dim slice of Wo's input.
Host sums the 4 partials per batch element (Megatron-style row-parallel Wo).

Device layouts (per core):
  qT/kT/vT  [1024, 2048] bf16   (input, transposed on host)
  wqT/wkT/wvT [1024, 256] bf16  (Wq[js].T etc)
  woT       [256, 1024] bf16    (Wo[:, js].T)
  maskT     [2048, 2048] bf16   (mask[0,0].T as 0.0/1.0)
  qpT/kpT   [256(j), 2048(s)]   (projections, transposed: j on partitions)
  vp        [2048(t), 4x65]     (natural layout; col 64 of each 65-block = 1.0
                                 -> attn@V matmul also produces softmax denom)
  P~        [t, s] = exp(scoresT/8) * maskT   (scoresT = K_h.T^T @ Q_h.T)
  attn out  [65(j+denom), s] -> normalized -> concatT [256(j), 2048(s)]
  out_p     [2048, 1024] f32 partial = concatT.T @ woT
"""

import sys

import numpy as np

try:
    import concourse.bass as bass
except ImportError:  # pragma: no cover
    sys.path.insert(0, "/opt/trn_rl_repo")
    import concourse.bass as bass

from concourse import bacc

import ml_dtypes

import concourse.tile as tile_mod
from concourse import mybir
from concourse.bass_utils import run_bass_kernel_spmd

BF16 = ml_dtypes.bfloat16
F32 = np.float32

B, S, D, H = 2, 2048, 1024, 16
DK = D // H            # 64
N_CORES = 8
HPC = 4                # heads per core
JC = HPC * DK          # 256 j-dims per core
SCALE = 1.0 / float(np.sqrt(DK))
NSB = S // 512         # 4 s-blocks
NC_T = S // 128        # 16 t-chunks
VROW = HPC * 65        # 260: [h0 64 | 1 | h1 64 | 1 | ...]

bf = mybir.dt.bfloat16
f32 = mybir.dt.float32


def _patch_drain():
    """This walrus build only accepts 1 sync-wait per instruction; the Tile
    exit drain carries one wait per pending proc. Split them across drains."""
    if getattr(tile_mod.TileContext, "_drain_patched", False):
        return
    import bass_rust

    def _drain_and_barrier(self, tick_clock, wait_clock):
        from concourse.tile import ScopedClock

        nc = self.nc
        drain_inst = nc.sync.drain()
        wait_clock.add_sem_waits(
            drain_inst.ins, ScopedClock({None: tick_clock.global_clock})
        )
        si = drain_inst.ins.sync_info
        waits = list(si.on_wait)
        if len(waits) > 1:
            drain_inst.ins.sync_info = bass_rust.SyncInfo(
                on_wait=[waits[0]], on_update=list(si.on_update)
            )
            for w in waits[1:]:
                d2 = nc.sync.drain()
                d2.ins.sync_info = bass_rust.SyncInfo(on_wait=[w], on_update=[])
        nc.all_engine_barrier()
        assert self.sems is not None
        popped = nc._tile_sem_poison_stack.pop()
        assert popped is self._sem_poison
        nc.clear_and_free_semaphores(list(self.sems.allocated().values()))
        nc.all_engine_barrier()

    tile_mod.TileContext._drain_and_barrier = _drain_and_barrier
    tile_mod.TileContext._drain_patched = True


def _emit(tc, T):
    nc = tc.nc
    Exp = mybir.ActivationFunctionType.Exp

    from contextlib import ExitStack

    with ExitStack() as ctx:
        persist = ctx.enter_context(tc.tile_pool(name="persist", bufs=1))

        # ---- weights / persistent tiles ----
        wq = persist.tile([128, 8 * JC], bf, tag="wq")
        wk = persist.tile([128, 8 * JC], bf, tag="wk")
        wv = persist.tile([128, 8 * JC], bf, tag="wv")
        for t, name in ((wq, "wqT"), (wk, "wkT"), (wv, "wvT")):
            nc.sync.dma_start(
                t[:].rearrange("p (c j) -> p c j", c=8),
                T[name][:, :].rearrange("(c p) j -> p c j", p=128),
            )
        wo = [persist.tile([128, D], bf, tag=f"wo{i}", name=f"wo{i}") for i in range(2)]
        for i in range(2):
            nc.sync.dma_start(wo[i][:], T["woT"][i * 128 : (i + 1) * 128, :])
        biasqk = persist.tile([128, 4], f32, tag="biasqk")
        nc.sync.dma_start(biasqk[:], T["biasqk"][:, :])

        # per-sb q/k projection tiles ([j, s] transposed layout)
        qpS = [
            [persist.tile([128, 512], bf, tag=f"qp{j}_{s}", name=f"qp{j}_{s}")
             for s in range(NSB)]
            for j in range(2)
        ]
        kpT = [
            [persist.tile([128, 1024], bf, tag=f"kpT{i}_{th}", name=f"kpT{i}_{th}")
             for th in range(2)]
            for i in range(2)
        ]
        # per-chunk v tiles (natural [t, j] layout + ones cols)
        vpc = [persist.tile([128, VROW], bf, tag=f"vp{c}", name=f"vp{c}")
               for c in range(NC_T)]
        concatT = [persist.tile([128, S], bf, tag=f"concatT{i}", name=f"concatT{i}") for i in range(2)]

        wq_v = wq[:].rearrange("p (c j) -> p c j", c=8)
        wk_v = wk[:].rearrange("p (c j) -> p c j", c=8)
        wv_v = wv[:].rearrange("p (c j) -> p c j", c=8)

        q_stream = ctx.enter_context(tc.tile_pool(name="q_stream", bufs=1))
        qtts = {}

        def emit_qdma(sb):
            sl = slice(sb * 512, (sb + 1) * 512)
            qTt = q_stream.tile([128, 8 * 512], bf, tag="qTt", name=f"qTt{sb}")
            nc.sync.dma_start(
                qTt[:].rearrange("p (c s) -> p c s", c=8),
                T["qT"][:, sl].rearrange("(c p) s -> p c s", p=128),
            )
            qtts[sb] = qTt[:].rearrange("p (c s) -> p c s", c=8)

        def emit_qproj_jt(sb, jt):
            jsl = slice(jt * 128, (jt + 1) * 128)
            ps = bigp.tile([128, 512], f32, tag="big", name=f"pq{sb}_{jt}")
            for c in range(8):
                nc.tensor.matmul(
                    ps[:], wq_v[:, c, jsl], qtts[sb][:, c, :],
                    start=(c == 0), stop=(c == 7),
                )
            nc.vector.tensor_scalar_add(
                qpS[jt][sb][:], ps[:], biasqk[:, jt : jt + 1]
            )

        def emit_qproj(sb):
            emit_qdma(sb)
            emit_qproj_jt(sb, 0)
            emit_qproj_jt(sb, 1)

        # ---- attention + output projection ----
        # Chunk-level software pipeline: per t-chunk the PE stream carries
        # scores(i) for both heads (concurrent row-groups), then attnV(i-1)
        # for both heads, plus occasional "extras" (Wo / q-proj / v-proj /
        # mask prefetch). ACT (exp) is the pacing engine; this keeps it fed
        # every chunk while the PE stays dense enough to hold HAM at 8/8.
        if True:
            vstream = ctx.enter_context(tc.tile_pool(name="vstream", bufs=2))
            maskp = ctx.enter_context(tc.tile_pool(name="maskp", bufs=2))
            ptp = ctx.enter_context(tc.tile_pool(name="ptp", bufs=2))
            smallp = ctx.enter_context(tc.tile_pool(name="smallp", bufs=2))
            outp = ctx.enter_context(tc.tile_pool(name="outp", bufs=2))
            scp = ctx.enter_context(tc.tile_pool(name="scp", bufs=2, space="PSUM"))
            mtiles = {}

        # ---- q(0) + k projections (scores need all of kpT) ----
        bigp = ctx.enter_context(tc.tile_pool(name="bigp", bufs=4, space="PSUM"))
        emit_qproj(0)
        with tc.tile_pool(name="kv_stream", bufs=2) as kv_stream:
            for sb in range(NSB):
                sl = slice(sb * 512, (sb + 1) * 512)
                kTt = kv_stream.tile([128, 8 * 512], bf, tag="kTt", name=f"kTt{sb}")
                nc.sync.dma_start(
                    kTt[:].rearrange("p (c s) -> p c s", c=8),
                    T["kT"][:, sl].rearrange("(c p) s -> p c s", p=128),
                )
                kTt_v = kTt[:].rearrange("p (c s) -> p c s", c=8)
                for jt in range(2):
                    jsl = slice(jt * 128, (jt + 1) * 128)
                    ps = bigp.tile([128, 512], f32, tag="big", name=f"pk{sb}_{jt}")
                    for c in range(8):
                        nc.tensor.matmul(
                            ps[:], wk_v[:, c, jsl], kTt_v[:, c, :],
                            start=(c == 0), stop=(c == 7),
                        )
                    nc.vector.tensor_scalar_add(
                        kpT[jt][sb // 2][:, (sb % 2) * 512 : (sb % 2 + 1) * 512],
                        ps[:], biasqk[:, 2 + jt : 3 + jt]
                    )




            def emit_mask_dma(sb):
                sl = slice(sb * 512, (sb + 1) * 512)
                mT = maskp.tile([128, NC_T * 512], bf, tag="mT", name=f"mT{sb}")
                nc.gpsimd.dma_start(
                    mT[:].rearrange("p (c s) -> p c s", c=NC_T),
                    T["maskT"][:, sl].rearrange("(c p) s -> p c s", p=128),
                )
                mtiles[sb] = mT

            vtts = {}

            def emit_vdma(tb):
                sl = slice(tb * 512, (tb + 1) * 512)
                vTt = vstream.tile([128, 8 * 512], bf, tag="vTt", name=f"vTt{tb}")
                nc.gpsimd.dma_start(
                    vTt[:].rearrange("p (c s) -> p c s", c=8),
                    T["vT"][:, sl].rearrange("(c p) s -> p c s", p=128),
                )
                vtts[tb] = vTt[:].rearrange("p (c t) -> p c t", c=8)

            def emit_vproj_tb(tb):
                vTt_v = vtts[tb]
                if tb + 2 < NSB:
                    emit_vdma(tb + 2)
                for tt in range(4):
                    chunk = tb * 4 + tt
                    ps = bigp.tile([128, 512], f32, tag="big", name=f"pv{chunk}")
                    for c in range(8):
                        nc.tensor.matmul(
                            ps[:, 0:JC],
                            vTt_v[:, c, tt * 128 : (tt + 1) * 128],
                            wv_v[:, c, :],
                            start=(c == 0), stop=(c == 7),
                        )
                    vt = vpc[chunk]
                    nc.gpsimd.memset(
                        vt[:].rearrange("p (h d) -> p h d", d=65)[:, :, 64:65],
                        1.0,
                    )
                    dst = vt[:].rearrange("p (h d) -> p h d", h=HPC)[:, :, 0:DK]
                    src = ps[:, 0:JC].rearrange("p (h d) -> p h d", h=HPC)
                    nc.vector.tensor_copy(dst, src)

            def emit_wo_group(sb, st, mt):
                s0 = sb * 512 + st * 128
                msl = slice(mt * 512, (mt + 1) * 512)
                pw = bigp.tile([128, 512], f32, tag="big", name=f"pw{sb}_{st}_{mt}")
                for kc in range(2):
                    nc.tensor.matmul(
                        pw[:],
                        concatT[kc][:, s0 : s0 + 128],
                        wo[kc][:, msl],
                        start=(kc == 0), stop=(kc == 1),
                    )
                ot = outp.tile([128, 512], f32, tag="ot", name=f"ot{sb}_{st}_{mt}")
                nc.vector.tensor_copy(ot[:], pw[:])
                nc.sync.dma_start(T["out_p"][s0 : s0 + 128, msl], ot[:])

            def emit_norm(sb, pair, po2):
                sl = slice(sb * 512, (sb + 1) * 512)
                for h2 in range(2):
                    h = pair * 2 + h2
                    psl = slice(h2 * 64, h2 * 64 + 64)
                    po = po2[h2]
                    rc0 = smallp.tile([1, 512], f32, tag="rc0", name=f"rc0_{sb}_{h}")
                    nc.vector.tensor_copy(rc0[:], po[64:65, :])
                    rc = smallp.tile([1, 512], f32, tag="rc", name=f"rc{sb}_{h}")
                    nc.vector.reciprocal_approx_fast(rc[:], rc0[:])
                    rb = smallp.tile([64, 512], f32, tag="rb", name=f"rb{sb}_{h}")
                    nc.gpsimd.partition_broadcast(rb[:], rc[:], channels=64)
                    nc.vector.tensor_mul(
                        concatT[pair][psl, sl], po[0:64, :], rb[:]
                    )

            emit_mask_dma(0)
            emit_vdma(0)
            emit_vdma(1)
            extras = []
            po2L = None
            prev = None        # (sb, pair, Pt, po2)
            for sb in range(NSB):
                for pair in range(2):
                    last_it = (sb == NSB - 1 and pair == 1)
                    if sb == 0:
                        extras.append(lambda t=2 * pair: emit_vproj_tb(t))
                        extras.append(lambda t=2 * pair + 1: emit_vproj_tb(t))

                    Pt = ptp.tile(
                        [128, 2 * NC_T * 512], bf, tag="Pt", name=f"Pt{sb}_{pair}"
                    )
                    pv = Pt[:].rearrange("p (c h s) -> p c h s", c=NC_T, h=2)
                    mv = mtiles[sb][:].rearrange("p (c s) -> p c s", c=NC_T)
                    if prev is not None:
                        po2 = [
                            bigp.tile([128, 512], f32, tag="big",
                                      name=f"av{prev[0]}_{prev[1]}_{h2}")
                            for h2 in range(2)
                        ]
                    for c in range(NC_T):
                        ps = scp.tile(
                            [128, 1024], f32, tag="sc", name=f"sc{sb}_{pair}_{c}"
                        )
                        for h2 in range(2):
                            psl = slice(h2 * 64, h2 * 64 + 64)
                            nc.tensor.matmul(
                                ps[:, h2 * 512 : (h2 + 1) * 512],
                                kpT[pair][c // 8][psl, (c % 8) * 128 : (c % 8 + 1) * 128],
                                qpS[pair][sb][psl, :],
                                start=True, stop=True,
                            )
                        nc.scalar.activation(
                            Pt[:, c * 1024 : (c + 1) * 1024],
                            ps[:], Exp, scale=SCALE,
                        )
                        if prev is not None:
                            psb, ppair, pPt, _ = prev
                            for h2 in range(2):
                                h = ppair * 2 + h2
                                nc.tensor.matmul(
                                    po2[h2][0:65, :],
                                    vpc[c][:, h * 65 : h * 65 + 65],
                                    pPt[:, (2 * c + h2) * 512 : (2 * c + h2 + 1) * 512],
                                    start=(c == 0), stop=(c == NC_T - 1),
                                )
                        if last_it and c >= 8:
                            if c == 8:
                                po2L = [
                                    bigp.tile([128, 512], f32, tag="big",
                                              name=f"avL_{h2}")
                                    for h2 in range(2)
                                ]
                            cc = c - 8
                            for h2 in range(2):
                                h = pair * 2 + h2
                                nc.tensor.matmul(
                                    po2L[h2][0:65, :],
                                    vpc[cc][:, h * 65 : h * 65 + 65],
                                    Pt[:, (2 * cc + h2) * 512 : (2 * cc + h2 + 1) * 512],
                                    start=(cc == 0), stop=False,
                                )
                        if c == 7 or c == NC_T - 1:
                            half = slice(0, 8) if c == 7 else slice(8, NC_T)
                            for h2 in range(2):
                                nc.vector.tensor_mul(
                                    pv[:, half, h2, :], pv[:, half, h2, :],
                                    mv[:, half, :],
                                )
                        if c == 1 and pair == 0 and sb + 1 < NSB:
                            emit_mask_dma(sb + 1)
                            emit_qdma(sb + 1)
                        elif c in (1, 3) and pair == 1 and sb + 1 < NSB:
                            emit_qproj_jt(sb + 1, c // 2)
                        elif extras and c % 2 == 1 and (c >= 5 or (pair == 0 and c >= 3)):
                            extras.pop(0)()
                    if prev is not None:
                        emit_norm(prev[0], prev[1], po2)
                        if prev[1] == 1:
                            for st in range(4):
                                for mt in range(2):
                                    extras.append(
                                        lambda s=prev[0], a=st, b=mt:
                                        emit_wo_group(s, a, b)
                                    )
                    prev = (sb, pair, Pt, None)
            # tail: finish attnv(3,1) chunks 8..15, then norm + final Wo
            psb, ppair, pPt, _ = prev
            for c in range(8, NC_T):
                for h2 in range(2):
                    h = ppair * 2 + h2
                    nc.tensor.matmul(
                        po2L[h2][0:65, :],
                        vpc[c][:, h * 65 : h * 65 + 65],
                        pPt[:, (2 * c + h2) * 512 : (2 * c + h2 + 1) * 512],
                        start=False, stop=(c == NC_T - 1),
                    )
                if extras and c % 2 == 1:
                    extras.pop(0)()
            emit_norm(psb, ppair, po2L)
            for fn in extras:
                fn()
            for st in range(4):
                for mt in range(2):
                    emit_wo_group(NSB - 1, st, mt)


def build_nc():
    nc = bacc.Bacc("TRN2", target_bir_lowering=False, debug=False)
    names = {}
    def din(name, shape, dt):
        names[name] = nc.dram_tensor(name, shape, dt, kind="ExternalInput").ap()
    din("qT", [D, S], bf)
    din("kT", [D, S], bf)
    din("vT", [D, S], bf)
    din("maskT", [S, S], bf)
    din("wqT", [D, JC], bf)
    din("wkT", [D, JC], bf)
    din("wvT", [D, JC], bf)
    din("woT", [JC, D], bf)
    din("biasqk", [128, 4], f32)
    names["out_p"] = nc.dram_tensor(
        "out_p", [S, D], f32, kind="ExternalOutput"
    ).ap()
    with tile_mod.TileContext(nc) as tc:
        _emit(tc, names)
    nc.compile()
    return nc


_NC = None


def prep_inputs(q, k, v, mask, Wq, bq, Wk, bk, Wv, bv, Wo, bo):
    q = np.asarray(q, F32)
    k = np.asarray(k, F32)
    v = np.asarray(v, F32)
    mask = np.asarray(mask)
    Wq, Wk, Wv, Wo = (np.asarray(w, F32) for w in (Wq, Wk, Wv, Wo))
    bq, bk, bv, bo = (np.asarray(b_, F32) for b_ in (bq, bk, bv, bo))

    maskT = np.ascontiguousarray(mask[0, 0].T).astype(BF16)
    qT = [np.ascontiguousarray(q[b_].T).astype(BF16) for b_ in range(B)]
    kT = [np.ascontiguousarray(k[b_].T).astype(BF16) for b_ in range(B)]
    vT = [np.ascontiguousarray(v[b_].T).astype(BF16) for b_ in range(B)]

    in_maps = []
    for c in range(N_CORES):
        b_, g = c // 4, c % 4
        js = slice(g * JC, (g + 1) * JC)
        biasqk = np.stack(
            [bq[js][:128], bq[js][128:], bk[js][:128], bk[js][128:]], axis=1
        ).astype(F32)
        in_maps.append(
            {
                "qT": qT[b_],
                "kT": kT[b_],
                "vT": vT[b_],
                "maskT": maskT,
                "wqT": np.ascontiguousarray(Wq[js, :].T).astype(BF16),
                "wkT": np.ascontiguousarray(Wk[js, :].T).astype(BF16),
                "wvT": np.ascontiguousarray(Wv[js, :].T).astype(BF16),
                "woT": np.ascontiguousarray(Wo[:, js].T).astype(BF16),
                "biasqk": np.ascontiguousarray(biasqk),
            }
        )
    # bv contributes a constant (softmax rows sum to 1): out += Wo @ bv + bo
    bias_out = (Wo @ bv + bo).astype(F32)
    return in_maps, bias_out


def run_prepped(in_maps, bias_out, trace=False, **kw):
    global _NC
    if _NC is None:
        _NC = build_nc()
    res = run_bass_kernel_spmd(
        _NC, in_maps, list(range(N_CORES)), trace=trace, **kw
    )
    out = np.zeros((B, S, D), F32)
    for c in range(N_CORES):
        out[c // 4] += res.results[c]["out_p"]
    out += bias_out[None, None, :]
    return out, res


def kernel(q, k, v, mask, Wq, bq, Wk, bk, Wv, bv, Wo, bo):
    in_maps, bias_out = prep_inputs(
        q, k, v, mask, Wq, bq, Wk, bk, Wv, bv, Wo, bo
    )
    out, _ = run_prepped(in_maps, bias_out)
    return out



# revision 9
# speedup vs baseline: 1.0566x; 1.0566x over previous
"""Trainium2 Bass kernel for nn_MultiHeadAttention (B=2, S=2048, D=1024, H=16).

Sharding: 8 cores = 2 (batch) x 4 (head groups of 4 heads / 256 dims).
Each core computes QKV projections for its head slice, attention for its 4
heads, and the partial output projection for its 256-dim slice of Wo's input.
Host sums the 4 partials per batch element (Megatron-style row-parallel Wo).

Device layouts (per core):
  qT/kT/vT  [1024, 2048] bf16   (input, transposed on host)
  wqT/wkT/wvT [1024, 256] bf16  (Wq[js].T etc)
  woT       [256, 1024] bf16    (Wo[:, js].T)
  maskT     [2048, 2048] bf16   (mask[0,0].T as 0.0/1.0)
  qpT/kpT   [256(j), 2048(s)]   (projections, transposed: j on partitions)
  vp        [2048(t), 4x65]     (natural layout; col 64 of each 65-block = 1.0
                                 -> attn@V matmul also produces softmax denom)
  P~        [t, s] = exp(scoresT/8) * maskT   (scoresT = K_h.T^T @ Q_h.T)
  attn out  [65(j+denom), s] -> normalized -> concatT [256(j), 2048(s)]
  out_p     [2048, 1024] f32 partial = concatT.T @ woT
"""

import sys

import numpy as np

try:
    import concourse.bass as bass
except ImportError:  # pragma: no cover
    sys.path.insert(0, "/opt/trn_rl_repo")
    import concourse.bass as bass

from concourse import bacc

import ml_dtypes

import concourse.tile as tile_mod
from concourse import mybir
from concourse.bass_utils import run_bass_kernel_spmd

BF16 = ml_dtypes.bfloat16
F32 = np.float32

B, S, D, H = 2, 2048, 1024, 16
DK = D // H            # 64
N_CORES = 8
HPC = 4                # heads per core
JC = HPC * DK          # 256 j-dims per core
SCALE = 1.0 / float(np.sqrt(DK))
NSB = S // 512         # 4 s-blocks
NC_T = S // 128        # 16 t-chunks
VROW = HPC * 65        # 260: [h0 64 | 1 | h1 64 | 1 | ...]

bf = mybir.dt.bfloat16
f32 = mybir.dt.float32


def _patch_drain():
    """This walrus build only accepts 1 sync-wait per instruction; the Tile
    exit drain carries one wait per pending proc. Split them across drains."""
    if getattr(tile_mod.TileContext, "_drain_patched", False):
        return
    import bass_rust

    def _drain_and_barrier(self, tick_clock, wait_clock):
        from concourse.tile import ScopedClock

        nc = self.nc
        drain_inst = nc.sync.drain()
        wait_clock.add_sem_waits(
            drain_inst.ins, ScopedClock({None: tick_clock.global_clock})
        )
        si = drain_inst.ins.sync_info
        waits = list(si.on_wait)
        if len(waits) > 1:
            drain_inst.ins.sync_info = bass_rust.SyncInfo(
                on_wait=[waits[0]], on_update=list(si.on_update)
            )
            for w in waits[1:]:
                d2 = nc.sync.drain()
                d2.ins.sync_info = bass_rust.SyncInfo(on_wait=[w], on_update=[])
        nc.all_engine_barrier()
        assert self.sems is not None
        popped = nc._tile_sem_poison_stack.pop()
        assert popped is self._sem_poison
        nc.clear_and_free_semaphores(list(self.sems.allocated().values()))
        nc.all_engine_barrier()

    tile_mod.TileContext._drain_and_barrier = _drain_and_barrier
    tile_mod.TileContext._drain_patched = True


def _emit(tc, T):
    nc = tc.nc
    Exp = mybir.ActivationFunctionType.Exp

    from contextlib import ExitStack

    with ExitStack() as ctx:
        persist = ctx.enter_context(tc.tile_pool(name="persist", bufs=1))

        # ---- weights / persistent tiles ----
        wq = persist.tile([128, 8 * JC], bf, tag="wq")
        wk = persist.tile([128, 8 * JC], bf, tag="wk")
        wv = persist.tile([128, 8 * JC], bf, tag="wv")
        wo = [persist.tile([128, D], bf, tag=f"wo{i}", name=f"wo{i}") for i in range(2)]
        biasqk = persist.tile([128, 4], f32, tag="biasqk")

        def emit_wdma(t, name):
            nc.sync.dma_start(
                t[:].rearrange("p (c j) -> p c j", c=8),
                T[name][:, :].rearrange("(c p) j -> p c j", p=128),
            )

        def emit_wodma(i):
            nc.sync.dma_start(wo[i][:], T["woT"][i * 128 : (i + 1) * 128, :])

        # per-sb q/k projection tiles ([j, s] transposed layout)
        qpS = [
            [persist.tile([128, 512], bf, tag=f"qp{j}_{s}", name=f"qp{j}_{s}")
             for s in range(NSB)]
            for j in range(2)
        ]
        kpT = [
            [persist.tile([128, 1024], bf, tag=f"kpT{i}_{th}", name=f"kpT{i}_{th}")
             for th in range(2)]
            for i in range(2)
        ]
        # per-chunk v tiles (natural [t, j] layout + ones cols)
        vpc = [persist.tile([128, VROW], bf, tag=f"vp{c}", name=f"vp{c}")
               for c in range(NC_T)]
        concatT = [persist.tile([128, S], bf, tag=f"concatT{i}", name=f"concatT{i}") for i in range(2)]

        wq_v = wq[:].rearrange("p (c j) -> p c j", c=8)
        wk_v = wk[:].rearrange("p (c j) -> p c j", c=8)
        wv_v = wv[:].rearrange("p (c j) -> p c j", c=8)

        q_stream = ctx.enter_context(tc.tile_pool(name="q_stream", bufs=1))
        kv_stream = ctx.enter_context(tc.tile_pool(name="kv_stream", bufs=2))
        vstream = ctx.enter_context(tc.tile_pool(name="vstream", bufs=2))
        maskp = ctx.enter_context(tc.tile_pool(name="maskp", bufs=2))
        ptp = ctx.enter_context(tc.tile_pool(name="ptp", bufs=2))
        smallp = ctx.enter_context(tc.tile_pool(name="smallp", bufs=2))
        outp = ctx.enter_context(tc.tile_pool(name="outp", bufs=2))
        scp = ctx.enter_context(tc.tile_pool(name="scp", bufs=2, space="PSUM"))
        bigp = ctx.enter_context(tc.tile_pool(name="bigp", bufs=4, space="PSUM"))
        mtiles = {}
        qtts = {}
        ktts = {}
        vtts = {}

        def emit_qdma(sb):
            sl = slice(sb * 512, (sb + 1) * 512)
            qTt = q_stream.tile([128, 8 * 512], bf, tag="qTt", name=f"qTt{sb}")
            nc.sync.dma_start(
                qTt[:].rearrange("p (c s) -> p c s", c=8),
                T["qT"][:, sl].rearrange("(c p) s -> p c s", p=128),
            )
            qtts[sb] = qTt[:].rearrange("p (c s) -> p c s", c=8)

        def emit_qproj_jt(sb, jt):
            jsl = slice(jt * 128, (jt + 1) * 128)
            ps = bigp.tile([128, 512], f32, tag="big", name=f"pq{sb}_{jt}")
            for c in range(8):
                nc.tensor.matmul(
                    ps[:], wq_v[:, c, jsl], qtts[sb][:, c, :],
                    start=(c == 0), stop=(c == 7),
                )
            nc.vector.tensor_scalar_add(
                qpS[jt][sb][:], ps[:], biasqk[:, jt : jt + 1]
            )

        def emit_kdma(sb):
            sl = slice(sb * 512, (sb + 1) * 512)
            kTt = kv_stream.tile([128, 8 * 512], bf, tag="kTt", name=f"kTt{sb}")
            nc.sync.dma_start(
                kTt[:].rearrange("p (c s) -> p c s", c=8),
                T["kT"][:, sl].rearrange("(c p) s -> p c s", p=128),
            )
            ktts[sb] = kTt[:].rearrange("p (c s) -> p c s", c=8)

        def emit_kproj_jt(sb, jt):
            jsl = slice(jt * 128, (jt + 1) * 128)
            ps = bigp.tile([128, 512], f32, tag="big", name=f"pk{sb}_{jt}")
            for c in range(8):
                nc.tensor.matmul(
                    ps[:], wk_v[:, c, jsl], ktts[sb][:, c, :],
                    start=(c == 0), stop=(c == 7),
                )
            nc.vector.tensor_scalar_add(
                kpT[jt][sb // 2][:, (sb % 2) * 512 : (sb % 2 + 1) * 512],
                ps[:], biasqk[:, 2 + jt : 3 + jt]
            )

        def emit_mask_dma(sb):
            sl = slice(sb * 512, (sb + 1) * 512)
            mT = maskp.tile([128, NC_T * 512], bf, tag="mT", name=f"mT{sb}")
            nc.gpsimd.dma_start(
                mT[:].rearrange("p (c s) -> p c s", c=NC_T),
                T["maskT"][:, sl].rearrange("(c p) s -> p c s", p=128),
            )
            mtiles[sb] = mT

        def emit_vdma(tb):
            sl = slice(tb * 512, (tb + 1) * 512)
            vTt = vstream.tile([128, 8 * 512], bf, tag="vTt", name=f"vTt{tb}")
            nc.gpsimd.dma_start(
                vTt[:].rearrange("p (c s) -> p c s", c=8),
                T["vT"][:, sl].rearrange("(c p) s -> p c s", p=128),
            )
            vtts[tb] = vTt[:].rearrange("p (c t) -> p c t", c=8)

        def emit_vproj(chunk):
            tb, tt = chunk // 4, chunk % 4
            vTt_v = vtts[tb]
            ps = bigp.tile([128, 512], f32, tag="big", name=f"pv{chunk}")
            for c in range(8):
                nc.tensor.matmul(
                    ps[:, 0:JC],
                    vTt_v[:, c, tt * 128 : (tt + 1) * 128],
                    wv_v[:, c, :],
                    start=(c == 0), stop=(c == 7),
                )
            vt = vpc[chunk]
            nc.gpsimd.memset(
                vt[:].rearrange("p (h d) -> p h d", d=65)[:, :, 64:65],
                1.0,
            )
            dst = vt[:].rearrange("p (h d) -> p h d", h=HPC)[:, :, 0:DK]
            src = ps[:, 0:JC].rearrange("p (h d) -> p h d", h=HPC)
            nc.vector.tensor_copy(dst, src)

        def emit_wo_group(sb, st, mt):
            s0 = sb * 512 + st * 128
            msl = slice(mt * 512, (mt + 1) * 512)
            pw = bigp.tile([128, 512], f32, tag="big", name=f"pw{sb}_{st}_{mt}")
            for kc in range(2):
                nc.tensor.matmul(
                    pw[:],
                    concatT[kc][:, s0 : s0 + 128],
                    wo[kc][:, msl],
                    start=(kc == 0), stop=(kc == 1),
                )
            ot = outp.tile([128, 512], f32, tag="ot", name=f"ot{sb}_{st}_{mt}")
            nc.vector.tensor_copy(ot[:], pw[:])
            nc.sync.dma_start(T["out_p"][s0 : s0 + 128, msl], ot[:])

        def emit_norm(sb, pair, po2):
            sl = slice(sb * 512, (sb + 1) * 512)
            for h2 in range(2):
                h = pair * 2 + h2
                psl = slice(h2 * 64, h2 * 64 + 64)
                po = po2[h2]
                rc0 = smallp.tile([1, 512], f32, tag="rc0", name=f"rc0_{sb}_{h}")
                nc.vector.tensor_copy(rc0[:], po[64:65, :])
                rc = smallp.tile([1, 512], f32, tag="rc", name=f"rc{sb}_{h}")
                nc.vector.reciprocal_approx_fast(rc[:], rc0[:])
                rb = smallp.tile([64, 512], f32, tag="rb", name=f"rb{sb}_{h}")
                nc.gpsimd.partition_broadcast(rb[:], rc[:], channels=64)
                nc.vector.tensor_mul(
                    concatT[pair][psl, sl], po[0:64, :], rb[:]
                )

        # ---- static extras schedule ----
        # extras[it][c] -> list of thunks, emitted after that chunk's
        # scores+exp+attnV.  Placement is deadline-driven: a producer must be
        # EMITTED strictly before the first chunk whose instructions consume
        # it (the PE queue is in-order; a consumer emitted earlier would
        # head-of-line block on data its own queue never produces).
        extras = {it: {} for it in range(8)}

        def sched(it, c, fn):
            extras[it].setdefault(c, []).append(fn)

        # it0 (0,0): k projections (j0 feeds this iteration chunk 4s'..;
        # j1 feeds it1), v projections (feed attnV during it1), q j1.
        # kv_stream bufs=2: both projections of kTt(s) must be emitted
        # before emit_kdma(s+2) recycles the buffer.
        sched(0, 1, lambda: emit_kproj_jt(0, 1))
        sched(0, 2, lambda: emit_kproj_jt(1, 0))
        sched(0, 3, lambda: emit_kdma(2))
        sched(0, 4, lambda: emit_kproj_jt(1, 1))
        sched(0, 5, lambda: emit_kdma(3))
        sched(0, 5, lambda: emit_vproj(0))
        sched(0, 6, lambda: emit_kproj_jt(2, 0))
        sched(0, 7, lambda: emit_vproj(1))
        sched(0, 8, lambda: emit_kproj_jt(2, 1))
        sched(0, 8, lambda: emit_wodma(0))
        sched(0, 9, lambda: emit_vproj(2))
        sched(0, 10, lambda: emit_kproj_jt(3, 0))
        sched(0, 11, lambda: emit_vproj(3))
        sched(0, 12, lambda: emit_kproj_jt(3, 1))
        sched(0, 12, lambda: emit_vdma(2))
        sched(0, 13, lambda: emit_vproj(4))
        sched(0, 14, lambda: emit_qproj_jt(0, 1))
        sched(0, 14, lambda: emit_wodma(1))
        sched(0, 15, lambda: emit_vproj(5))
        # it1 (0,1): vproj 6..15, prefetch q(1)/mask(1)
        for i, cc in enumerate(range(1, 11)):
            sched(1, cc, lambda ch=6 + i: emit_vproj(ch))
        sched(1, 3, lambda: emit_vdma(3))
        sched(1, 8, lambda: emit_qdma(1))
        sched(1, 12, lambda: emit_qproj_jt(1, 0))
        sched(1, 14, lambda: emit_mask_dma(1))
        # steady iterations
        for sb in range(1, NSB):
            it = 2 * sb
            sched(it, 0, lambda s=sb: emit_qproj_jt(s, 1))
            # wo for previous sb: its concatT is only complete after
            # norm(sb-1, 1), which runs at the END of iteration (sb, 0) —
            # so the wo groups go in iteration (sb, 1).
            for g in range(8):
                sched(it + 1, g,
                      lambda s=sb - 1, a=g // 2, b=g % 2: emit_wo_group(s, a, b))
            if sb + 1 < NSB:
                sched(it, 12, lambda s=sb + 1: emit_qdma(s))
                sched(it, 14, lambda s=sb + 1: emit_mask_dma(s))
                sched(it + 1, 12, lambda s=sb + 1: emit_qproj_jt(s, 0))

        # ---- prologue ----
        nc.sync.dma_start(biasqk[:], T["biasqk"][:, :])
        emit_wdma(wk, "wkT")
        emit_kdma(0)
        emit_wdma(wq, "wqT")
        emit_qdma(0)
        emit_kdma(1)
        emit_wdma(wv, "wvT")
        emit_vdma(0)
        emit_mask_dma(0)
        emit_vdma(1)
        emit_kproj_jt(0, 0)
        emit_qproj_jt(0, 0)

        # ---- main pipeline ----
        # Per chunk: attnV(i-1) first (deps always stale -> PE never
        # head-of-line blocks), then scores(i) (waits only on the exp two
        # chunks back), then exp on ACT.  Extras fill the remaining PE slack.
        po2L = None
        prev = None        # (sb, pair, Pt)
        for sb in range(NSB):
            for pair in range(2):
                it = 2 * sb + pair
                last_it = (sb == NSB - 1 and pair == 1)

                Pt = ptp.tile(
                    [128, 2 * NC_T * 512], bf, tag="Pt", name=f"Pt{sb}_{pair}"
                )
                pv = Pt[:].rearrange("p (c h s) -> p c h s", c=NC_T, h=2)
                mv = mtiles[sb][:].rearrange("p (c s) -> p c s", c=NC_T)
                if prev is not None:
                    po2 = [
                        bigp.tile([128, 512], f32, tag="big",
                                  name=f"av{prev[0]}_{prev[1]}_{h2}")
                        for h2 in range(2)
                    ]
                for c in range(NC_T):
                    if prev is not None:
                        psb, ppair, pPt = prev
                        for h2 in range(2):
                            h = ppair * 2 + h2
                            nc.tensor.matmul(
                                po2[h2][0:65, :],
                                vpc[c][:, h * 65 : h * 65 + 65],
                                pPt[:, (2 * c + h2) * 512 : (2 * c + h2 + 1) * 512],
                                start=(c == 0), stop=(c == NC_T - 1),
                            )
                    ps = scp.tile(
                        [128, 1024], f32, tag="sc", name=f"sc{sb}_{pair}_{c}"
                    )
                    for h2 in range(2):
                        psl = slice(h2 * 64, h2 * 64 + 64)
                        nc.tensor.matmul(
                            ps[:, h2 * 512 : (h2 + 1) * 512],
                            kpT[pair][c // 8][psl, (c % 8) * 128 : (c % 8 + 1) * 128],
                            qpS[pair][sb][psl, :],
                            start=True, stop=True,
                        )
                    nc.scalar.activation(
                        Pt[:, c * 1024 : (c + 1) * 1024],
                        ps[:], Exp, scale=SCALE,
                    )
                    if last_it and c >= 8:
                        if c == 8:
                            po2L = [
                                bigp.tile([128, 512], f32, tag="big",
                                          name=f"avL_{h2}")
                                for h2 in range(2)
                            ]
                        cc = c - 8
                        for h2 in range(2):
                            h = pair * 2 + h2
                            nc.tensor.matmul(
                                po2L[h2][0:65, :],
                                vpc[cc][:, h * 65 : h * 65 + 65],
                                Pt[:, (2 * cc + h2) * 512 : (2 * cc + h2 + 1) * 512],
                                start=(cc == 0), stop=False,
                            )
                    if c == 7 or c == NC_T - 1:
                        half = slice(0, 8) if c == 7 else slice(8, NC_T)
                        for h2 in range(2):
                            nc.vector.tensor_mul(
                                pv[:, half, h2, :], pv[:, half, h2, :],
                                mv[:, half, :],
                            )
                    for fn in extras[it].get(c, ()):
                        fn()
                if prev is not None:
                    emit_norm(prev[0], prev[1], po2)
                prev = (sb, pair, Pt)
        # tail: finish attnv(3,1) chunks 8..15, then norm + final Wo
        psb, ppair, pPt = prev
        for c in range(8, NC_T):
            for h2 in range(2):
                h = ppair * 2 + h2
                nc.tensor.matmul(
                    po2L[h2][0:65, :],
                    vpc[c][:, h * 65 : h * 65 + 65],
                    pPt[:, (2 * c + h2) * 512 : (2 * c + h2 + 1) * 512],
                    start=False, stop=(c == NC_T - 1),
                )
        emit_norm(psb, ppair, po2L)
        for st in range(4):
            for mt in range(2):
                emit_wo_group(NSB - 1, st, mt)


def build_nc():
    nc = bacc.Bacc("TRN2", target_bir_lowering=False, debug=False)
    names = {}
    def din(name, shape, dt):
        names[name] = nc.dram_tensor(name, shape, dt, kind="ExternalInput").ap()
    din("qT", [D, S], bf)
    din("kT", [D, S], bf)
    din("vT", [D, S], bf)
    din("maskT", [S, S], bf)
    din("wqT", [D, JC], bf)
    din("wkT", [D, JC], bf)
    din("wvT", [D, JC], bf)
    din("woT", [JC, D], bf)
    din("biasqk", [128, 4], f32)
    names["out_p"] = nc.dram_tensor(
        "out_p", [S, D], f32, kind="ExternalOutput"
    ).ap()
    with tile_mod.TileContext(nc) as tc:
        _emit(tc, names)
    nc.compile()
    return nc


_NC = None


def prep_inputs(q, k, v, mask, Wq, bq, Wk, bk, Wv, bv, Wo, bo):
    q = np.asarray(q, F32)
    k = np.asarray(k, F32)
    v = np.asarray(v, F32)
    mask = np.asarray(mask)
    Wq, Wk, Wv, Wo = (np.asarray(w, F32) for w in (Wq, Wk, Wv, Wo))
    bq, bk, bv, bo = (np.asarray(b_, F32) for b_ in (bq, bk, bv, bo))

    maskT = np.ascontiguousarray(mask[0, 0].T).astype(BF16)
    qT = [np.ascontiguousarray(q[b_].T).astype(BF16) for b_ in range(B)]
    kT = [np.ascontiguousarray(k[b_].T).astype(BF16) for b_ in range(B)]
    vT = [np.ascontiguousarray(v[b_].T).astype(BF16) for b_ in range(B)]

    in_maps = []
    for c in range(N_CORES):
        b_, g = c // 4, c % 4
        js = slice(g * JC, (g + 1) * JC)
        biasqk = np.stack(
            [bq[js][:128], bq[js][128:], bk[js][:128], bk[js][128:]], axis=1
        ).astype(F32)
        in_maps.append(
            {
                "qT": qT[b_],
                "kT": kT[b_],
                "vT": vT[b_],
                "maskT": maskT,
                "wqT": np.ascontiguousarray(Wq[js, :].T).astype(BF16),
                "wkT": np.ascontiguousarray(Wk[js, :].T).astype(BF16),
                "wvT": np.ascontiguousarray(Wv[js, :].T).astype(BF16),
                "woT": np.ascontiguousarray(Wo[:, js].T).astype(BF16),
                "biasqk": np.ascontiguousarray(biasqk),
            }
        )
    # bv contributes a constant (softmax rows sum to 1): out += Wo @ bv + bo
    bias_out = (Wo @ bv + bo).astype(F32)
    return in_maps, bias_out


def run_prepped(in_maps, bias_out, trace=False, **kw):
    global _NC
    if _NC is None:
        _NC = build_nc()
    res = run_bass_kernel_spmd(
        _NC, in_maps, list(range(N_CORES)), trace=trace, **kw
    )
    out = np.zeros((B, S, D), F32)
    for c in range(N_CORES):
        out[c // 4] += res.results[c]["out_p"]
    out += bias_out[None, None, :]
    return out, res


def kernel(q, k, v, mask, Wq, bq, Wk, bk, Wv, bv, Wo, bo):
    in_maps, bias_out = prep_inputs(
        q, k, v, mask, Wq, bq, Wk, bk, Wv, bv, Wo, bo
    )
    out, _ = run_prepped(in_maps, bias_out)
    return out



# revision 19
# speedup vs baseline: 1.0934x; 1.0348x over previous
"""Trainium2 Bass kernel for nn_MultiHeadAttention (B=2, S=2048, D=1024, H=16).

Sharding: 8 cores = 2 (batch) x 4 (head groups of 4 heads / 256 dims).
Each core computes QKV projections for its head slice, attention for its 4
heads, and the partial output projection for its 256-dim slice of Wo's input.
Host sums the 4 partials per batch element (Megatron-style row-parallel Wo).

Device layouts (per core):
  qT/kT/vT  [1024, 2048] bf16   (input, transposed on host)
  wqT/wkT/wvT [1024, 256] bf16  (Wq[js].T etc)
  woT       [256, 1024] bf16    (Wo[:, js].T)
  maskT     [2048, 2048] bf16   (mask[0,0].T as 0.0/1.0)
  qpT/kpT   [256(j), 2048(s)]   (projections, transposed: j on partitions)
  vp        [2048(t), 4x65]     (natural layout; col 64 of each 65-block = 1.0
                                 -> attn@V matmul also produces softmax denom)
  P~        [t, s] = exp(scoresT/8) * maskT   (scoresT = K_h.T^T @ Q_h.T)
  attn out  [65(j+denom), s] -> normalized -> concatT [256(j), 2048(s)]
  out_p     [2048, 1024] f32 partial = concatT.T @ woT
"""

import sys

import numpy as np

try:
    import concourse.bass as bass
except ImportError:  # pragma: no cover
    sys.path.insert(0, "/opt/trn_rl_repo")
    import concourse.bass as bass

from concourse import bacc

import ml_dtypes

import concourse.tile as tile_mod
from concourse import mybir
from concourse.bass_utils import run_bass_kernel_spmd

BF16 = ml_dtypes.bfloat16
F32 = np.float32

B, S, D, H = 2, 2048, 1024, 16
DK = D // H            # 64
N_CORES = 8
HPC = 4                # heads per core
JC = HPC * DK          # 256 j-dims per core
SCALE = 1.0 / float(np.sqrt(DK))
NSB = S // 512         # 4 s-blocks
NC_T = S // 128        # 16 t-chunks
VROW = HPC * 65        # 260: [h0 64 | 1 | h1 64 | 1 | ...]

bf = mybir.dt.bfloat16
f32 = mybir.dt.float32


def _patch_drain():
    """This walrus build only accepts 1 sync-wait per instruction; the Tile
    exit drain carries one wait per pending proc. Split them across drains."""
    if getattr(tile_mod.TileContext, "_drain_patched", False):
        return
    import bass_rust

    def _drain_and_barrier(self, tick_clock, wait_clock):
        from concourse.tile import ScopedClock

        nc = self.nc
        drain_inst = nc.sync.drain()
        wait_clock.add_sem_waits(
            drain_inst.ins, ScopedClock({None: tick_clock.global_clock})
        )
        si = drain_inst.ins.sync_info
        waits = list(si.on_wait)
        if len(waits) > 1:
            drain_inst.ins.sync_info = bass_rust.SyncInfo(
                on_wait=[waits[0]], on_update=list(si.on_update)
            )
            for w in waits[1:]:
                d2 = nc.sync.drain()
                d2.ins.sync_info = bass_rust.SyncInfo(on_wait=[w], on_update=[])
        nc.all_engine_barrier()
        assert self.sems is not None
        popped = nc._tile_sem_poison_stack.pop()
        assert popped is self._sem_poison
        nc.clear_and_free_semaphores(list(self.sems.allocated().values()))
        nc.all_engine_barrier()

    tile_mod.TileContext._drain_and_barrier = _drain_and_barrier
    tile_mod.TileContext._drain_patched = True


def _emit(tc, T):
    nc = tc.nc
    Exp = mybir.ActivationFunctionType.Exp

    from contextlib import ExitStack

    with ExitStack() as ctx:
        persist = ctx.enter_context(tc.tile_pool(name="persist", bufs=1))

        # ---- weights / persistent tiles ----
        wq = persist.tile([128, 8 * JC], bf, tag="wq")
        wk = persist.tile([128, 8 * JC], bf, tag="wk")
        wv = persist.tile([128, 8 * JC], bf, tag="wv")
        wo = [persist.tile([128, D], bf, tag=f"wo{i}", name=f"wo{i}") for i in range(2)]
        biasqk = persist.tile([128, 4], f32, tag="biasqk")

        def emit_wdma(t, name):
            # host pre-tiles weights as [128, 8, JC] so each partition's
            # 8*JC*2B run is contiguous (big DMA descriptors)
            nc.sync.dma_start(
                t[:].rearrange("p (c j) -> p c j", c=8),
                T[name][:, :, :],
            )

        def emit_wodma(i):
            nc.sync.dma_start(wo[i][:], T["woT"][i * 128 : (i + 1) * 128, :])

        # per-sb q/k projection tiles ([j, s] transposed layout)
        qpS = [
            [persist.tile([128, 512], bf, tag=f"qp{j}_{s}", name=f"qp{j}_{s}")
             for s in range(NSB)]
            for j in range(2)
        ]
        kpT = [
            [persist.tile([128, 1024], bf, tag=f"kpT{i}_{th}", name=f"kpT{i}_{th}")
             for th in range(2)]
            for i in range(2)
        ]
        # per-chunk v tiles (natural [t, j] layout + ones cols)
        vpc = [persist.tile([128, VROW], bf, tag=f"vp{c}", name=f"vp{c}")
               for c in range(NC_T)]
        concatT = [persist.tile([128, S], bf, tag=f"concatT{i}", name=f"concatT{i}") for i in range(2)]

        wq_v = wq[:].rearrange("p (c j) -> p c j", c=8)
        wk_v = wk[:].rearrange("p (c j) -> p c j", c=8)
        wv_v = wv[:].rearrange("p (c j) -> p c j", c=8)

        q_stream = ctx.enter_context(tc.tile_pool(name="q_stream", bufs=1))
        kv_stream = ctx.enter_context(tc.tile_pool(name="kv_stream", bufs=2))
        vstream = ctx.enter_context(tc.tile_pool(name="vstream", bufs=2))
        maskp = ctx.enter_context(tc.tile_pool(name="maskp", bufs=2))
        ptp = ctx.enter_context(tc.tile_pool(name="ptp", bufs=2))
        smallp = ctx.enter_context(tc.tile_pool(name="smallp", bufs=2))
        outp = ctx.enter_context(tc.tile_pool(name="outp", bufs=2))
        scp = ctx.enter_context(tc.tile_pool(name="scp", bufs=2, space="PSUM"))
        bigp = ctx.enter_context(tc.tile_pool(name="bigp", bufs=4, space="PSUM"))
        mtiles = {}
        qtts = {}
        ktts = {}
        vtts = {}
        otiles = {}

        def emit_qdma(sb):
            qTt = q_stream.tile([128, 8 * 512], bf, tag="qTt", name=f"qTt{sb}")
            nc.sync.dma_start(
                qTt[:].rearrange("p (c s) -> p c s", c=8),
                T["qT"][sb, :, :, :],
            )
            qtts[sb] = qTt[:].rearrange("p (c s) -> p c s", c=8)

        def emit_qproj_jt(sb, jt):
            jsl = slice(jt * 128, (jt + 1) * 128)
            ps = bigp.tile([128, 512], f32, tag="big", name=f"pq{sb}_{jt}")
            for c in range(8):
                nc.tensor.matmul(
                    ps[:], wq_v[:, c, jsl], qtts[sb][:, c, :],
                    start=(c == 0), stop=(c == 7),
                )
            nc.vector.tensor_scalar_add(
                qpS[jt][sb][:], ps[:], biasqk[:, jt : jt + 1]
            )

        def emit_kdma(sb):
            kTt = kv_stream.tile([128, 8 * 512], bf, tag="kTt", name=f"kTt{sb}")
            nc.sync.dma_start(
                kTt[:].rearrange("p (c s) -> p c s", c=8),
                T["kT"][sb, :, :, :],
            )
            ktts[sb] = kTt[:].rearrange("p (c s) -> p c s", c=8)

        def emit_kproj_jt(sb, jt):
            jsl = slice(jt * 128, (jt + 1) * 128)
            ps = bigp.tile([128, 512], f32, tag="big", name=f"pk{sb}_{jt}")
            for c in range(8):
                nc.tensor.matmul(
                    ps[:], wk_v[:, c, jsl], ktts[sb][:, c, :],
                    start=(c == 0), stop=(c == 7),
                )
            nc.vector.tensor_scalar_add(
                kpT[jt][sb // 2][:, (sb % 2) * 512 : (sb % 2 + 1) * 512],
                ps[:], biasqk[:, 2 + jt : 3 + jt]
            )

        def emit_mask_dma(sb):
            mT = maskp.tile([128, NC_T * 512], bf, tag="mT", name=f"mT{sb}")
            nc.gpsimd.dma_start(
                mT[:].rearrange("p (c s) -> p c s", c=NC_T),
                T["maskT"][sb, :, :, :],
            )
            mtiles[sb] = mT

        def emit_vdma(tb):
            vTt = vstream.tile([128, 8 * 512], bf, tag="vTt", name=f"vTt{tb}")
            nc.gpsimd.dma_start(
                vTt[:].rearrange("p (c s) -> p c s", c=8),
                T["vT"][tb, :, :, :],
            )
            vtts[tb] = vTt[:].rearrange("p (c t) -> p c t", c=8)

        def emit_vproj(chunk):
            tb, tt = chunk // 4, chunk % 4
            vTt_v = vtts[tb]
            ps = bigp.tile([128, 512], f32, tag="big", name=f"pv{chunk}")
            for c in range(8):
                nc.tensor.matmul(
                    ps[:, 0:JC],
                    vTt_v[:, c, tt * 128 : (tt + 1) * 128],
                    wv_v[:, c, :],
                    start=(c == 0), stop=(c == 7),
                )
            vt = vpc[chunk]
            nc.gpsimd.memset(
                vt[:].rearrange("p (h d) -> p h d", d=65)[:, :, 64:65],
                1.0,
            )
            dst = vt[:].rearrange("p (h d) -> p h d", h=HPC)[:, :, 0:DK]
            src = ps[:, 0:JC].rearrange("p (h d) -> p h d", h=HPC)
            nc.vector.tensor_copy(dst, src)

        def emit_wo_group(sb, st, mt):
            # out partial in bf16 (summed in fp32 on host); the (sb, st, 1)
            # group also fires the single full-row DMA for both halves.
            s0 = sb * 512 + st * 128
            msl = slice(mt * 512, (mt + 1) * 512)
            pw = bigp.tile([128, 512], f32, tag="big", name=f"pw{sb}_{st}_{mt}")
            for kc in range(2):
                nc.tensor.matmul(
                    pw[:],
                    concatT[kc][:, s0 : s0 + 128],
                    wo[kc][:, msl],
                    start=(kc == 0), stop=(kc == 1),
                )
            if mt == 0:
                otiles[(sb, st)] = outp.tile(
                    [128, 1024], bf, tag="ot", name=f"ot{sb}_{st}"
                )
            ot = otiles[(sb, st)]
            nc.vector.tensor_copy(ot[:, msl], pw[:])
            if mt == 1:
                nc.sync.dma_start(T["out_p"][s0 : s0 + 128, :], ot[:])

        def emit_norm(sb, pair, po2):
            sl = slice(sb * 512, (sb + 1) * 512)
            for h2 in range(2):
                h = pair * 2 + h2
                psl = slice(h2 * 64, h2 * 64 + 64)
                po = po2[h2]
                rc0 = smallp.tile([1, 512], f32, tag="rc0", name=f"rc0_{sb}_{h}")
                nc.vector.tensor_copy(rc0[:], po[64:65, :])
                rc = smallp.tile([1, 512], f32, tag="rc", name=f"rc{sb}_{h}")
                nc.vector.reciprocal_approx_fast(rc[:], rc0[:])
                rb = smallp.tile([64, 512], f32, tag="rb", name=f"rb{sb}_{h}")
                nc.gpsimd.partition_broadcast(rb[:], rc[:], channels=64)
                nc.vector.tensor_mul(
                    concatT[pair][psl, sl], po[0:64, :], rb[:]
                )

        # ---- static extras schedule ----
        # extras[it][c] -> list of thunks, emitted after that chunk's
        # scores+exp+attnV.  Placement is deadline-driven: a producer must be
        # EMITTED strictly before the first chunk whose instructions consume
        # it (the PE queue is in-order; a consumer emitted earlier would
        # head-of-line block on data its own queue never produces).
        extras = {it: {} for it in range(8)}

        def sched(it, c, fn):
            extras[it].setdefault(c, []).append(fn)

        # it0 (0,0): k projections (j0 feeds this iteration chunk 4s'..;
        # j1 feeds it1), v projections (feed attnV during it1), q j1.
        # kv_stream bufs=2: both projections of kTt(s) must be emitted
        # before emit_kdma(s+2) recycles the buffer.
        sched(0, 1, lambda: emit_kproj_jt(0, 1))
        sched(0, 2, lambda: emit_kproj_jt(1, 0))
        sched(0, 3, lambda: emit_kdma(2))
        sched(0, 4, lambda: emit_kproj_jt(1, 1))
        sched(0, 5, lambda: emit_kdma(3))
        sched(0, 5, lambda: emit_vproj(0))
        sched(0, 6, lambda: emit_kproj_jt(2, 0))
        sched(0, 7, lambda: emit_vproj(1))
        sched(0, 8, lambda: emit_kproj_jt(2, 1))
        sched(0, 8, lambda: emit_wodma(0))
        sched(0, 9, lambda: emit_vproj(2))
        sched(0, 10, lambda: emit_kproj_jt(3, 0))
        sched(0, 11, lambda: emit_vproj(3))
        sched(0, 12, lambda: emit_kproj_jt(3, 1))
        sched(0, 12, lambda: emit_vdma(2))
        sched(0, 13, lambda: emit_vproj(4))
        sched(0, 14, lambda: emit_qproj_jt(0, 1))
        sched(0, 14, lambda: emit_wodma(1))
        sched(0, 15, lambda: emit_vproj(5))
        # it1 (0,1): vproj 6..15, prefetch q(1)/mask(1)
        for i, cc in enumerate(range(1, 11)):
            sched(1, cc, lambda ch=6 + i: emit_vproj(ch))
        sched(1, 3, lambda: emit_vdma(3))
        sched(1, 8, lambda: emit_qdma(1))
        sched(1, 12, lambda: emit_qproj_jt(1, 0))
        sched(1, 14, lambda: emit_mask_dma(1))
        # steady iterations
        for sb in range(1, NSB):
            it = 2 * sb
            sched(it, 0, lambda s=sb: emit_qproj_jt(s, 1))
            # wo for previous sb: its concatT is only complete after
            # norm(sb-1, 1), which runs at the END of iteration (sb, 0) —
            # so the wo groups go in iteration (sb, 1).
            for g in range(8):
                sched(it + 1, g,
                      lambda s=sb - 1, a=g // 2, b=g % 2: emit_wo_group(s, a, b))
            if sb + 1 < NSB:
                sched(it, 12, lambda s=sb + 1: emit_qdma(s))
                sched(it, 14, lambda s=sb + 1: emit_mask_dma(s))
                sched(it + 1, 12, lambda s=sb + 1: emit_qproj_jt(s, 0))

        # ---- prologue ----
        nc.sync.dma_start(biasqk[:], T["biasqk"][:, :])
        emit_wdma(wk, "wkT")
        emit_kdma(0)
        emit_wdma(wq, "wqT")
        emit_qdma(0)
        emit_kdma(1)
        emit_wdma(wv, "wvT")
        emit_mask_dma(0)
        emit_vdma(0)
        emit_vdma(1)
        # HAM warm-up: ~4us of dummy matmuls while the first DMAs land, so
        # the real projections start at the 2.4 GHz clock (K=8/8).
        warm = persist.tile([128, 512], bf, tag="warm")
        nc.gpsimd.memset(warm[:], 0.0)
        wps = bigp.tile([128, 512], f32, tag="big", name="warmps")
        for i in range(10):
            nc.tensor.matmul(
                wps[:], warm[:, 0:128], warm[:],
                start=(i == 0), stop=(i == 9),
            )
        emit_kproj_jt(0, 0)
        emit_qproj_jt(0, 0)

        # ---- main pipeline ----
        # Per chunk: attnV(i-1) first (deps always stale -> PE never
        # head-of-line blocks), then scores(i) (waits only on the exp two
        # chunks back), then exp on ACT.  Extras fill the remaining PE slack.
        po2L = None
        prev = None        # (sb, pair, Pt)
        for sb in range(NSB):
            for pair in range(2):
                it = 2 * sb + pair
                last_it = (sb == NSB - 1 and pair == 1)

                Pt = ptp.tile(
                    [128, 2 * NC_T * 512], bf, tag="Pt", name=f"Pt{sb}_{pair}"
                )
                pv = Pt[:].rearrange("p (c h s) -> p c h s", c=NC_T, h=2)
                mv = mtiles[sb][:].rearrange("p (c s) -> p c s", c=NC_T)
                if prev is not None:
                    po2 = [
                        bigp.tile([128, 512], f32, tag="big",
                                  name=f"av{prev[0]}_{prev[1]}_{h2}")
                        for h2 in range(2)
                    ]
                for c in range(NC_T):
                    if prev is not None:
                        psb, ppair, pPt = prev
                        for h2 in range(2):
                            h = ppair * 2 + h2
                            nc.tensor.matmul(
                                po2[h2][0:65, :],
                                vpc[c][:, h * 65 : h * 65 + 65],
                                pPt[:, (2 * c + h2) * 512 : (2 * c + h2 + 1) * 512],
                                start=(c == 0), stop=(c == NC_T - 1),
                            )
                    ps = scp.tile(
                        [128, 1024], f32, tag="sc", name=f"sc{sb}_{pair}_{c}"
                    )
                    for h2 in range(2):
                        psl = slice(h2 * 64, h2 * 64 + 64)
                        nc.tensor.matmul(
                            ps[:, h2 * 512 : (h2 + 1) * 512],
                            kpT[pair][c // 8][psl, (c % 8) * 128 : (c % 8 + 1) * 128],
                            qpS[pair][sb][psl, :],
                            start=True, stop=True,
                        )
                    nc.scalar.activation(
                        Pt[:, c * 1024 : (c + 1) * 1024],
                        ps[:], Exp, scale=SCALE,
                    )
                    if last_it and c >= 8:
                        if c == 8:
                            po2L = [
                                bigp.tile([128, 512], f32, tag="big",
                                          name=f"avL_{h2}")
                                for h2 in range(2)
                            ]
                        cc = c - 8
                        for h2 in range(2):
                            h = pair * 2 + h2
                            nc.tensor.matmul(
                                po2L[h2][0:65, :],
                                vpc[cc][:, h * 65 : h * 65 + 65],
                                Pt[:, (2 * cc + h2) * 512 : (2 * cc + h2 + 1) * 512],
                                start=(cc == 0), stop=False,
                            )
                    if c == 7 or c == NC_T - 1:
                        half = slice(0, 8) if c == 7 else slice(8, NC_T)
                        for h2 in range(2):
                            nc.vector.tensor_mul(
                                pv[:, half, h2, :], pv[:, half, h2, :],
                                mv[:, half, :],
                            )
                    for fn in extras[it].get(c, ()):
                        fn()
                if prev is not None:
                    emit_norm(prev[0], prev[1], po2)
                prev = (sb, pair, Pt)
        # tail: finish attnv(3,1) chunks 8..15, then norm + final Wo
        psb, ppair, pPt = prev
        for c in range(8, NC_T):
            for h2 in range(2):
                h = ppair * 2 + h2
                nc.tensor.matmul(
                    po2L[h2][0:65, :],
                    vpc[c][:, h * 65 : h * 65 + 65],
                    pPt[:, (2 * c + h2) * 512 : (2 * c + h2 + 1) * 512],
                    start=False, stop=(c == NC_T - 1),
                )
        emit_norm(psb, ppair, po2L)
        for st in range(4):
            for mt in range(2):
                emit_wo_group(NSB - 1, st, mt)


def build_nc():
    nc = bacc.Bacc("TRN2", target_bir_lowering=False, debug=False)
    names = {}
    def din(name, shape, dt):
        names[name] = nc.dram_tensor(name, shape, dt, kind="ExternalInput").ap()
    # q/k/v pre-tiled on host to [sb, p, c, s] and mask to [sb, p, c, s] so
    # every DMA descriptor covers a full 8-16KB partition line (the
    # descriptor-generation rate, ~12ns/descriptor, caps DMA throughput
    # otherwise).
    din("qT", [NSB, 128, 8, 512], bf)
    din("kT", [NSB, 128, 8, 512], bf)
    din("vT", [NSB, 128, 8, 512], bf)
    din("maskT", [NSB, 128, NC_T, 512], bf)
    din("wqT", [128, 8, JC], bf)
    din("wkT", [128, 8, JC], bf)
    din("wvT", [128, 8, JC], bf)
    din("woT", [JC, D], bf)
    din("biasqk", [128, 4], f32)
    names["out_p"] = nc.dram_tensor(
        "out_p", [S, D], bf, kind="ExternalOutput"
    ).ap()
    with tile_mod.TileContext(nc) as tc:
        _emit(tc, names)
    nc.compile()
    return nc


_NC = None


def _tile_ds(xT, nc_):
    """[D, S] -> [NSB, 128, nc_, S // nc_ // ...] host pre-tiling.

    Element (sb, p, c, s) = xT[c * 128 + p, sb * blk + s] where blk = S/NSB.
    """
    d, s_ = xT.shape
    blk = s_ // NSB
    nch = d // 128
    # xT[(c p), (sb s)] -> [c, p, sb, s] -> [sb, p, c, s]
    r = xT.reshape(nch, 128, NSB, blk).transpose(2, 1, 0, 3)
    return np.ascontiguousarray(r)


def prep_inputs(q, k, v, mask, Wq, bq, Wk, bk, Wv, bv, Wo, bo):
    q = np.asarray(q, F32)
    k = np.asarray(k, F32)
    v = np.asarray(v, F32)
    mask = np.asarray(mask)
    Wq, Wk, Wv, Wo = (np.asarray(w, F32) for w in (Wq, Wk, Wv, Wo))
    bq, bk, bv, bo = (np.asarray(b_, F32) for b_ in (bq, bk, bv, bo))

    maskT = _tile_ds(np.ascontiguousarray(mask[0, 0].T).astype(BF16), NC_T)
    qT = [_tile_ds(q[b_].T.astype(BF16), 8) for b_ in range(B)]
    kT = [_tile_ds(k[b_].T.astype(BF16), 8) for b_ in range(B)]
    vT = [_tile_ds(v[b_].T.astype(BF16), 8) for b_ in range(B)]

    def _tile_w(wT):
        # [D, JC] -> [128, 8, JC]
        return np.ascontiguousarray(
            wT.reshape(8, 128, JC).transpose(1, 0, 2)
        )

    in_maps = []
    for c in range(N_CORES):
        b_, g = c // 4, c % 4
        js = slice(g * JC, (g + 1) * JC)
        biasqk = np.stack(
            [bq[js][:128], bq[js][128:], bk[js][:128], bk[js][128:]], axis=1
        ).astype(F32)
        in_maps.append(
            {
                "qT": qT[b_],
                "kT": kT[b_],
                "vT": vT[b_],
                "maskT": maskT,
                "wqT": _tile_w(Wq[js, :].T.astype(BF16)),
                "wkT": _tile_w(Wk[js, :].T.astype(BF16)),
                "wvT": _tile_w(Wv[js, :].T.astype(BF16)),
                "woT": np.ascontiguousarray(Wo[:, js].T).astype(BF16),
                "biasqk": np.ascontiguousarray(biasqk),
            }
        )
    # bv contributes a constant (softmax rows sum to 1): out += Wo @ bv + bo
    bias_out = (Wo @ bv + bo).astype(F32)
    return in_maps, bias_out


def run_prepped(in_maps, bias_out, trace=False, **kw):
    global _NC
    if _NC is None:
        _NC = build_nc()
    res = run_bass_kernel_spmd(
        _NC, in_maps, list(range(N_CORES)), trace=trace, **kw
    )
    out = np.zeros((B, S, D), F32)
    for c in range(N_CORES):
        out[c // 4] += res.results[c]["out_p"].astype(F32)
    out += bias_out[None, None, :]
    return out, res


def kernel(q, k, v, mask, Wq, bq, Wk, bk, Wv, bv, Wo, bo):
    in_maps, bias_out = prep_inputs(
        q, k, v, mask, Wq, bq, Wk, bk, Wv, bv, Wo, bo
    )
    out, _ = run_prepped(in_maps, bias_out)
    return out



# revision 30
# speedup vs baseline: 1.1552x; 1.0565x over previous
"""Trainium2 Bass kernel for nn_MultiHeadAttention (B=2, S=2048, D=1024, H=16).

Sharding: 8 cores = 2 (batch) x 4 (head groups of 4 heads / 256 dims).
Each core computes QKV projections for its head slice, attention for its 4
heads, and the partial output projection for its 256-dim slice of Wo's input.
Host sums the 4 partials per batch element (Megatron-style row-parallel Wo).

Device layouts (per core):
  qT/kT/vT  [1024, 2048] bf16   (input, transposed on host)
  wqT/wkT/wvT [1024, 256] bf16  (Wq[js].T etc)
  woT       [256, 1024] bf16    (Wo[:, js].T)
  maskT     [2048, 2048] bf16   (mask[0,0].T as 0.0/1.0)
  qpT/kpT   [256(j), 2048(s)]   (projections, transposed: j on partitions)
  vp        [2048(t), 4x65]     (natural layout; col 64 of each 65-block = 1.0
                                 -> attn@V matmul also produces softmax denom)
  P~        [t, s] = exp(scoresT/8) * maskT   (scoresT = K_h.T^T @ Q_h.T)
  attn out  [65(j+denom), s] -> normalized -> concatT [256(j), 2048(s)]
  out_p     [2048, 1024] f32 partial = concatT.T @ woT
"""

import sys

import numpy as np

try:
    import concourse.bass as bass
except ImportError:  # pragma: no cover
    sys.path.insert(0, "/opt/trn_rl_repo")
    import concourse.bass as bass

from concourse import bacc

import ml_dtypes

import concourse.tile as tile_mod
from concourse import mybir
from concourse.bass_utils import run_bass_kernel_spmd

BF16 = ml_dtypes.bfloat16
F32 = np.float32

B, S, D, H = 2, 2048, 1024, 16
DK = D // H            # 64
N_CORES = 8
HPC = 4                # heads per core
JC = HPC * DK          # 256 j-dims per core
SCALE = 1.0 / float(np.sqrt(DK))
NSB = S // 512         # 4 s-blocks
NC_T = S // 128        # 16 t-chunks
VROW = HPC * 65        # 260: [h0 64 | 1 | h1 64 | 1 | ...]

bf = mybir.dt.bfloat16
f32 = mybir.dt.float32


def _patch_drain():
    """This walrus build only accepts 1 sync-wait per instruction; the Tile
    exit drain carries one wait per pending proc. Split them across drains."""
    if getattr(tile_mod.TileContext, "_drain_patched", False):
        return
    import bass_rust

    def _drain_and_barrier(self, tick_clock, wait_clock):
        from concourse.tile import ScopedClock

        nc = self.nc
        drain_inst = nc.sync.drain()
        wait_clock.add_sem_waits(
            drain_inst.ins, ScopedClock({None: tick_clock.global_clock})
        )
        si = drain_inst.ins.sync_info
        waits = list(si.on_wait)
        if len(waits) > 1:
            drain_inst.ins.sync_info = bass_rust.SyncInfo(
                on_wait=[waits[0]], on_update=list(si.on_update)
            )
            for w in waits[1:]:
                d2 = nc.sync.drain()
                d2.ins.sync_info = bass_rust.SyncInfo(on_wait=[w], on_update=[])
        nc.all_engine_barrier()
        assert self.sems is not None
        popped = nc._tile_sem_poison_stack.pop()
        assert popped is self._sem_poison
        nc.clear_and_free_semaphores(list(self.sems.allocated().values()))
        nc.all_engine_barrier()

    tile_mod.TileContext._drain_and_barrier = _drain_and_barrier
    tile_mod.TileContext._drain_patched = True


def _emit(tc, T):
    nc = tc.nc
    Exp = mybir.ActivationFunctionType.Exp

    from contextlib import ExitStack

    with ExitStack() as ctx:
        persist = ctx.enter_context(tc.tile_pool(name="persist", bufs=1))

        # ---- weights / persistent tiles ----
        wq = persist.tile([128, 8 * JC], bf, tag="wq")
        wk = persist.tile([128, 8 * JC], bf, tag="wk")
        wv = persist.tile([128, 8 * JC], bf, tag="wv")
        wo = [persist.tile([128, D], bf, tag=f"wo{i}", name=f"wo{i}") for i in range(2)]
        biasqk = persist.tile([128, 4], f32, tag="biasqk")

        # The HWDGE (sync-queue) path sustains only ~30-60 GB/s per
        # instruction; the SWDGE (gpsimd-queue) path measures ~150-200 GB/s.
        # Startup-critical transfers go on gpsimd, slack ones on sync.
        def emit_wdma(t, name, eng):
            # host pre-tiles weights as [128, 8, JC] so each partition's
            # 8*JC*2B run is contiguous (big DMA descriptors)
            eng.dma_start(
                t[:].rearrange("p (c j) -> p c j", c=8),
                T[name][:, :, :],
            )

        def emit_wodma(i):
            nc.sync.dma_start(wo[i][:], T["woT"][i * 128 : (i + 1) * 128, :])

        # per-sb q/k projection tiles ([j, s] transposed layout)
        qpS = [
            [persist.tile([128, 512], bf, tag=f"qp{j}_{s}", name=f"qp{j}_{s}")
             for s in range(NSB)]
            for j in range(2)
        ]
        kpT = [
            [persist.tile([128, 1024], bf, tag=f"kpT{i}_{th}", name=f"kpT{i}_{th}")
             for th in range(2)]
            for i in range(2)
        ]
        # per-chunk v tiles (natural [t, j] layout + ones cols)
        vpc = [persist.tile([128, VROW], bf, tag=f"vp{c}", name=f"vp{c}")
               for c in range(NC_T)]
        concatT = [persist.tile([128, S], bf, tag=f"concatT{i}", name=f"concatT{i}") for i in range(2)]

        wq_v = wq[:].rearrange("p (c j) -> p c j", c=8)
        wk_v = wk[:].rearrange("p (c j) -> p c j", c=8)
        wv_v = wv[:].rearrange("p (c j) -> p c j", c=8)

        q_stream = ctx.enter_context(tc.tile_pool(name="q_stream", bufs=1))
        kv_stream = ctx.enter_context(tc.tile_pool(name="kv_stream", bufs=3))
        vstream = ctx.enter_context(tc.tile_pool(name="vstream", bufs=2))
        maskp = ctx.enter_context(tc.tile_pool(name="maskp", bufs=3))
        ptp = ctx.enter_context(tc.tile_pool(name="ptp", bufs=2))
        smallp = ctx.enter_context(tc.tile_pool(name="smallp", bufs=2))
        outp = ctx.enter_context(tc.tile_pool(name="outp", bufs=2))
        scp = ctx.enter_context(tc.tile_pool(name="scp", bufs=2, space="PSUM"))
        bigp = ctx.enter_context(tc.tile_pool(name="bigp", bufs=4, space="PSUM"))
        mtiles = {}
        qtts = {}
        ktts = {}
        vtts = {}
        otiles = {}

        def emit_qdma(sb, eng=None):
            qTt = q_stream.tile([128, 8 * 512], bf, tag="qTt", name=f"qTt{sb}")
            (eng or nc.sync).dma_start(
                qTt[:].rearrange("p (c s) -> p c s", c=8),
                T["qT"][sb, :, :, :],
            )
            qtts[sb] = qTt[:].rearrange("p (c s) -> p c s", c=8)

        def emit_qproj_jt(sb, jt):
            jsl = slice(jt * 128, (jt + 1) * 128)
            ps = bigp.tile([128, 512], f32, tag="big", name=f"pq{sb}_{jt}")
            for c in range(8):
                nc.tensor.matmul(
                    ps[:], wq_v[:, c, jsl], qtts[sb][:, c, :],
                    start=(c == 0), stop=(c == 7),
                )
            nc.vector.tensor_scalar_add(
                qpS[jt][sb][:], ps[:], biasqk[:, jt : jt + 1]
            )

        def emit_kdma(sb):
            kTt = kv_stream.tile([128, 8 * 512], bf, tag="kTt", name=f"kTt{sb}")
            nc.gpsimd.dma_start(
                kTt[:].rearrange("p (c s) -> p c s", c=8),
                T["kT"][sb, :, :, :],
            )
            ktts[sb] = kTt[:].rearrange("p (c s) -> p c s", c=8)

        def emit_kproj_jt(sb, jt):
            jsl = slice(jt * 128, (jt + 1) * 128)
            ps = bigp.tile([128, 512], f32, tag="big", name=f"pk{sb}_{jt}")
            for c in range(8):
                nc.tensor.matmul(
                    ps[:], wk_v[:, c, jsl], ktts[sb][:, c, :],
                    start=(c == 0), stop=(c == 7),
                )
            nc.vector.tensor_scalar_add(
                kpT[jt][sb // 2][:, (sb % 2) * 512 : (sb % 2 + 1) * 512],
                ps[:], biasqk[:, 2 + jt : 3 + jt]
            )

        def emit_mask_dma(sb, hf):
            mT = maskp.tile([128, 8 * 512], bf, tag="mT", name=f"mT{sb}_{hf}")
            nc.gpsimd.dma_start(
                mT[:].rearrange("p (c s) -> p c s", c=8),
                T["maskT"][sb, :, hf * 8 : (hf + 1) * 8, :],
            )
            mtiles[(sb, hf)] = mT

        def emit_vdma(tb, eng=None):
            vTt = vstream.tile([128, 8 * 512], bf, tag="vTt", name=f"vTt{tb}")
            (eng or nc.gpsimd).dma_start(
                vTt[:].rearrange("p (c s) -> p c s", c=8),
                T["vT"][tb, :, :, :],
            )
            vtts[tb] = vTt[:].rearrange("p (c t) -> p c t", c=8)

        def emit_vproj(chunk):
            tb, tt = chunk // 4, chunk % 4
            vTt_v = vtts[tb]
            ps = bigp.tile([128, 512], f32, tag="big", name=f"pv{chunk}")
            for c in range(8):
                nc.tensor.matmul(
                    ps[:, 0:JC],
                    vTt_v[:, c, tt * 128 : (tt + 1) * 128],
                    wv_v[:, c, :],
                    start=(c == 0), stop=(c == 7),
                )
            vt = vpc[chunk]
            nc.gpsimd.memset(
                vt[:].rearrange("p (h d) -> p h d", d=65)[:, :, 64:65],
                1.0,
            )
            dst = vt[:].rearrange("p (h d) -> p h d", h=HPC)[:, :, 0:DK]
            src = ps[:, 0:JC].rearrange("p (h d) -> p h d", h=HPC)
            nc.vector.tensor_copy(dst, src)

        def emit_wo_group(sb, st, mt):
            # out partial in bf16 (summed in fp32 on host); the (sb, st, 1)
            # group also fires the single full-row DMA for both halves.
            s0 = sb * 512 + st * 128
            msl = slice(mt * 512, (mt + 1) * 512)
            pw = bigp.tile([128, 512], f32, tag="big", name=f"pw{sb}_{st}_{mt}")
            for kc in range(2):
                nc.tensor.matmul(
                    pw[:],
                    concatT[kc][:, s0 : s0 + 128],
                    wo[kc][:, msl],
                    start=(kc == 0), stop=(kc == 1),
                )
            if mt == 0:
                otiles[(sb, st)] = outp.tile(
                    [128, 1024], bf, tag="ot", name=f"ot{sb}_{st}"
                )
            ot = otiles[(sb, st)]
            nc.vector.tensor_copy(ot[:, msl], pw[:])
            if mt == 1:
                nc.sync.dma_start(T["out_p"][s0 : s0 + 128, :], ot[:])

        def emit_norm(sb, pair, po2):
            sl = slice(sb * 512, (sb + 1) * 512)
            for h2 in range(2):
                h = pair * 2 + h2
                psl = slice(h2 * 64, h2 * 64 + 64)
                po = po2[h2]
                rc0 = smallp.tile([1, 512], f32, tag="rc0", name=f"rc0_{sb}_{h}")
                nc.vector.tensor_copy(rc0[:], po[64:65, :])
                rc = smallp.tile([1, 512], f32, tag="rc", name=f"rc{sb}_{h}")
                nc.vector.reciprocal_approx_fast(rc[:], rc0[:])
                rb = smallp.tile([64, 512], f32, tag="rb", name=f"rb{sb}_{h}")
                nc.gpsimd.partition_broadcast(rb[:], rc[:], channels=64)
                nc.vector.tensor_mul(
                    concatT[pair][psl, sl], po[0:64, :], rb[:]
                )

        # ---- static extras schedule ----
        # extras[it][c] -> list of thunks, emitted after that chunk's
        # scores+exp+attnV.  Placement is deadline-driven: a producer must be
        # EMITTED strictly before the first chunk whose instructions consume
        # it (the PE queue is in-order; a consumer emitted earlier would
        # head-of-line block on data its own queue never produces).
        extras = {it: {} for it in range(8)}

        def sched(it, c, fn):
            extras[it].setdefault(c, []).append(fn)

        # it0 (0,0): k projections (j0 feeds this iteration from chunk 4s';
        # j1 feeds it1), v projections (feed attnV during it1), q j1.
        sched(0, 1, lambda: emit_kproj_jt(1, 0))
        sched(0, 4, lambda: emit_kproj_jt(2, 0))
        sched(0, 5, lambda: emit_kproj_jt(1, 1))
        sched(0, 6, lambda: emit_kproj_jt(3, 0))
        sched(0, 7, lambda: emit_vproj(0))
        sched(0, 8, lambda: emit_kproj_jt(2, 1))
        sched(0, 9, lambda: emit_vproj(1))
        sched(0, 10, lambda: emit_kproj_jt(3, 1))
        sched(0, 11, lambda: emit_vproj(2))
        sched(0, 12, lambda: emit_qproj_jt(0, 1))
        sched(0, 13, lambda: emit_vproj(3))
        sched(0, 14, lambda: emit_vproj(4))
        sched(0, 14, lambda: emit_vdma(2))
        sched(0, 15, lambda: emit_vproj(5))
        # it1 (0,1): vproj 6..15, prefetch q(1)/mask(1)
        for i, cc in enumerate(range(1, 11)):
            sched(1, cc, lambda ch=6 + i: emit_vproj(ch))
        sched(1, 3, lambda: emit_vdma(3))
        sched(1, 8, lambda: emit_qdma(1))
        sched(1, 12, lambda: emit_qproj_jt(1, 0))
        sched(1, 14, lambda: emit_mask_dma(1, 0))
        sched(1, 15, lambda: emit_mask_dma(1, 1))
        # steady iterations
        for sb in range(1, NSB):
            it = 2 * sb
            sched(it, 0, lambda s=sb: emit_qproj_jt(s, 1))
            # wo for previous sb: its concatT is only complete after
            # norm(sb-1, 1), which runs at the END of iteration (sb, 0) —
            # so the wo groups go in iteration (sb, 1).
            for g in range(8):
                sched(it + 1, g,
                      lambda s=sb - 1, a=g // 2, b=g % 2: emit_wo_group(s, a, b))
            if sb + 1 < NSB:
                sched(it, 12, lambda s=sb + 1: emit_qdma(s))
                sched(it, 14, lambda s=sb + 1: emit_mask_dma(s, 0))
                sched(it, 15, lambda s=sb + 1: emit_mask_dma(s, 1))
                sched(it + 1, 12, lambda s=sb + 1: emit_qproj_jt(s, 0))

        # ---- prologue ----
        # Critical-path DMAs on the fast SWDGE queue, in deadline order;
        # wv/v0/v1 and the wo weights ride the sync queue in parallel.
        nc.gpsimd.dma_start(biasqk[:], T["biasqk"][:, :])
        emit_wdma(wk, "wkT", nc.gpsimd)
        emit_kdma(0)
        emit_wdma(wq, "wqT", nc.gpsimd)
        emit_qdma(0, nc.gpsimd)
        emit_kdma(1)
        emit_kdma(2)
        emit_wdma(wv, "wvT", nc.sync)
        emit_vdma(0, nc.sync)
        emit_vdma(1, nc.sync)
        emit_wodma(0)
        emit_wodma(1)
        # HAM warm-up: ~4us of dummy matmuls while the first DMAs land, so
        # the real projections start at the 2.4 GHz clock (K=8/8).
        warm = persist.tile([128, 512], bf, tag="warm")
        nc.gpsimd.memset(warm[:], 0.0)
        wps = bigp.tile([128, 512], f32, tag="big", name="warmps")
        for i in range(10):
            nc.tensor.matmul(
                wps[:], warm[:, 0:128], warm[:],
                start=(i == 0), stop=(i == 9),
            )
        emit_kproj_jt(0, 0)
        emit_kproj_jt(0, 1)
        emit_qproj_jt(0, 0)
        # kTt(3) recycles kTt(0)'s buffer: kp(0, *) must be emitted first
        emit_kdma(3)
        emit_mask_dma(0, 0)
        emit_mask_dma(0, 1)

        # ---- main pipeline ----
        # Per chunk: attnV(i-1) first (deps always stale -> PE never
        # head-of-line blocks), then scores(i) (waits only on the exp two
        # chunks back), then exp on ACT.  Extras fill the remaining PE slack.
        po2L = None
        prev = None        # (sb, pair, Pt)
        for sb in range(NSB):
            for pair in range(2):
                it = 2 * sb + pair
                last_it = (sb == NSB - 1 and pair == 1)

                Pt = ptp.tile(
                    [128, 2 * NC_T * 512], bf, tag="Pt", name=f"Pt{sb}_{pair}"
                )
                pv = Pt[:].rearrange("p (c h s) -> p c h s", c=NC_T, h=2)
                if prev is not None:
                    po2 = [
                        bigp.tile([128, 512], f32, tag="big",
                                  name=f"av{prev[0]}_{prev[1]}_{h2}")
                        for h2 in range(2)
                    ]
                for c in range(NC_T):
                    if prev is not None:
                        psb, ppair, pPt = prev
                        for h2 in range(2):
                            h = ppair * 2 + h2
                            nc.tensor.matmul(
                                po2[h2][0:65, :],
                                vpc[c][:, h * 65 : h * 65 + 65],
                                pPt[:, (2 * c + h2) * 512 : (2 * c + h2 + 1) * 512],
                                start=(c == 0), stop=(c == NC_T - 1),
                            )
                    ps = scp.tile(
                        [128, 1024], f32, tag="sc", name=f"sc{sb}_{pair}_{c}"
                    )
                    for h2 in range(2):
                        psl = slice(h2 * 64, h2 * 64 + 64)
                        nc.tensor.matmul(
                            ps[:, h2 * 512 : (h2 + 1) * 512],
                            kpT[pair][c // 8][psl, (c % 8) * 128 : (c % 8 + 1) * 128],
                            qpS[pair][sb][psl, :],
                            start=True, stop=True,
                        )
                    nc.scalar.activation(
                        Pt[:, c * 1024 : (c + 1) * 1024],
                        ps[:], Exp, scale=SCALE,
                    )
                    if last_it and c >= 8:
                        if c == 8:
                            po2L = [
                                bigp.tile([128, 512], f32, tag="big",
                                          name=f"avL_{h2}")
                                for h2 in range(2)
                            ]
                        cc = c - 8
                        for h2 in range(2):
                            h = pair * 2 + h2
                            nc.tensor.matmul(
                                po2L[h2][0:65, :],
                                vpc[cc][:, h * 65 : h * 65 + 65],
                                Pt[:, (2 * cc + h2) * 512 : (2 * cc + h2 + 1) * 512],
                                start=(cc == 0), stop=False,
                            )
                    if c == 7 or c == NC_T - 1:
                        hf = 0 if c == 7 else 1
                        half = slice(hf * 8, hf * 8 + 8)
                        mv = mtiles[(sb, hf)][:].rearrange(
                            "p (c s) -> p c s", c=8
                        )
                        for h2 in range(2):
                            nc.vector.tensor_mul(
                                pv[:, half, h2, :], pv[:, half, h2, :],
                                mv[:, :, :],
                            )
                    for fn in extras[it].get(c, ()):
                        fn()
                if prev is not None:
                    emit_norm(prev[0], prev[1], po2)
                prev = (sb, pair, Pt)
        # tail: finish attnv(3,1) chunks 8..15, then norm + final Wo
        psb, ppair, pPt = prev
        for c in range(8, NC_T):
            for h2 in range(2):
                h = ppair * 2 + h2
                nc.tensor.matmul(
                    po2L[h2][0:65, :],
                    vpc[c][:, h * 65 : h * 65 + 65],
                    pPt[:, (2 * c + h2) * 512 : (2 * c + h2 + 1) * 512],
                    start=False, stop=(c == NC_T - 1),
                )
        emit_norm(psb, ppair, po2L)
        for st in range(4):
            for mt in range(2):
                emit_wo_group(NSB - 1, st, mt)


def build_nc():
    nc = bacc.Bacc("TRN2", target_bir_lowering=False, debug=False)
    names = {}
    def din(name, shape, dt):
        names[name] = nc.dram_tensor(name, shape, dt, kind="ExternalInput").ap()
    # q/k/v pre-tiled on host to [sb, p, c, s] and mask to [sb, p, c, s] so
    # every DMA descriptor covers a full 8-16KB partition line (the
    # descriptor-generation rate, ~12ns/descriptor, caps DMA throughput
    # otherwise).
    din("qT", [NSB, 128, 8, 512], bf)
    din("kT", [NSB, 128, 8, 512], bf)
    din("vT", [NSB, 128, 8, 512], bf)
    din("maskT", [NSB, 128, NC_T, 512], bf)
    din("wqT", [128, 8, JC], bf)
    din("wkT", [128, 8, JC], bf)
    din("wvT", [128, 8, JC], bf)
    din("woT", [JC, D], bf)
    din("biasqk", [128, 4], f32)
    names["out_p"] = nc.dram_tensor(
        "out_p", [S, D], bf, kind="ExternalOutput"
    ).ap()
    with tile_mod.TileContext(nc) as tc:
        _emit(tc, names)
    nc.compile()
    return nc


_NC = None


def _tile_ds(xT, nc_):
    """[D, S] -> [NSB, 128, nc_, S // nc_ // ...] host pre-tiling.

    Element (sb, p, c, s) = xT[c * 128 + p, sb * blk + s] where blk = S/NSB.
    """
    d, s_ = xT.shape
    blk = s_ // NSB
    nch = d // 128
    # xT[(c p), (sb s)] -> [c, p, sb, s] -> [sb, p, c, s]
    r = xT.reshape(nch, 128, NSB, blk).transpose(2, 1, 0, 3)
    return np.ascontiguousarray(r)


def prep_inputs(q, k, v, mask, Wq, bq, Wk, bk, Wv, bv, Wo, bo):
    q = np.asarray(q, F32)
    k = np.asarray(k, F32)
    v = np.asarray(v, F32)
    mask = np.asarray(mask)
    Wq, Wk, Wv, Wo = (np.asarray(w, F32) for w in (Wq, Wk, Wv, Wo))
    bq, bk, bv, bo = (np.asarray(b_, F32) for b_ in (bq, bk, bv, bo))

    maskT = _tile_ds(np.ascontiguousarray(mask[0, 0].T).astype(BF16), NC_T)
    qT = [_tile_ds(q[b_].T.astype(BF16), 8) for b_ in range(B)]
    kT = [_tile_ds(k[b_].T.astype(BF16), 8) for b_ in range(B)]
    vT = [_tile_ds(v[b_].T.astype(BF16), 8) for b_ in range(B)]

    def _tile_w(wT):
        # [D, JC] -> [128, 8, JC]
        return np.ascontiguousarray(
            wT.reshape(8, 128, JC).transpose(1, 0, 2)
        )

    in_maps = []
    for c in range(N_CORES):
        b_, g = c // 4, c % 4
        js = slice(g * JC, (g + 1) * JC)
        biasqk = np.stack(
            [bq[js][:128], bq[js][128:], bk[js][:128], bk[js][128:]], axis=1
        ).astype(F32)
        in_maps.append(
            {
                "qT": qT[b_],
                "kT": kT[b_],
                "vT": vT[b_],
                "maskT": maskT,
                "wqT": _tile_w(Wq[js, :].T.astype(BF16)),
                "wkT": _tile_w(Wk[js, :].T.astype(BF16)),
                "wvT": _tile_w(Wv[js, :].T.astype(BF16)),
                "woT": np.ascontiguousarray(Wo[:, js].T).astype(BF16),
                "biasqk": np.ascontiguousarray(biasqk),
            }
        )
    # bv contributes a constant (softmax rows sum to 1): out += Wo @ bv + bo
    bias_out = (Wo @ bv + bo).astype(F32)
    return in_maps, bias_out


def run_prepped(in_maps, bias_out, trace=False, **kw):
    global _NC
    if _NC is None:
        _NC = build_nc()
    res = run_bass_kernel_spmd(
        _NC, in_maps, list(range(N_CORES)), trace=trace, **kw
    )
    out = np.zeros((B, S, D), F32)
    for c in range(N_CORES):
        out[c // 4] += res.results[c]["out_p"].astype(F32)
    out += bias_out[None, None, :]
    return out, res


def kernel(q, k, v, mask, Wq, bq, Wk, bk, Wv, bv, Wo, bo):
    in_maps, bias_out = prep_inputs(
        q, k, v, mask, Wq, bq, Wk, bk, Wv, bv, Wo, bo
    )
    out, _ = run_prepped(in_maps, bias_out)
    return out



# revision 47
# speedup vs baseline: 1.1727x; 1.0152x over previous
"""Trainium2 Bass kernel for nn_MultiHeadAttention (B=2, S=2048, D=1024, H=16).

Sharding: 8 cores = 2 (batch) x 4 (head groups of 4 heads / 256 dims).
Each core computes QKV projections for its head slice, attention for its 4
heads, and the partial output projection for its 256-dim slice of Wo's input.
Host sums the 4 partials per batch element (Megatron-style row-parallel Wo).

Device layouts (per core):
  qT/kT/vT  [1024, 2048] bf16   (input, transposed on host)
  wqT/wkT/wvT [1024, 256] bf16  (Wq[js].T etc)
  woT       [256, 1024] bf16    (Wo[:, js].T)
  maskT     [2048, 2048] bf16   (mask[0,0].T as 0.0/1.0)
  qpT/kpT   [256(j), 2048(s)]   (projections, transposed: j on partitions)
  vp        [2048(t), 4x65]     (natural layout; col 64 of each 65-block = 1.0
                                 -> attn@V matmul also produces softmax denom)
  P~        [t, s] = exp(scoresT/8) * maskT   (scoresT = K_h.T^T @ Q_h.T)
  attn out  [65(j+denom), s] -> normalized -> concatT [256(j), 2048(s)]
  out_p     [2048, 1024] f32 partial = concatT.T @ woT
"""

import sys

import numpy as np

try:
    import concourse.bass as bass
except ImportError:  # pragma: no cover
    sys.path.insert(0, "/opt/trn_rl_repo")
    import concourse.bass as bass

from concourse import bacc

import ml_dtypes

import concourse.tile as tile_mod
from concourse import mybir
from concourse.bass_utils import run_bass_kernel_spmd

BF16 = ml_dtypes.bfloat16
F32 = np.float32

B, S, D, H = 2, 2048, 1024, 16
DK = D // H            # 64
N_CORES = 8
HPC = 4                # heads per core
JC = HPC * DK          # 256 j-dims per core
SCALE = 1.0 / float(np.sqrt(DK))
NSB = S // 512         # 4 s-blocks
NC_T = S // 128        # 16 t-chunks
VROW = HPC * 128       # 512: [h0 64dims | 64 ones | h1 ...]; the 64
                       # ones-columns make attnV emit the softmax denom
                       # replicated on 64 psum partitions (free: matmul
                       # time is column-count of the moving operand)

bf = mybir.dt.bfloat16
f32 = mybir.dt.float32


def _patch_drain():
    """This walrus build only accepts 1 sync-wait per instruction; the Tile
    exit drain carries one wait per pending proc. Split them across drains."""
    if getattr(tile_mod.TileContext, "_drain_patched", False):
        return
    import bass_rust

    def _drain_and_barrier(self, tick_clock, wait_clock):
        from concourse.tile import ScopedClock

        nc = self.nc
        drain_inst = nc.sync.drain()
        wait_clock.add_sem_waits(
            drain_inst.ins, ScopedClock({None: tick_clock.global_clock})
        )
        si = drain_inst.ins.sync_info
        waits = list(si.on_wait)
        if len(waits) > 1:
            drain_inst.ins.sync_info = bass_rust.SyncInfo(
                on_wait=[waits[0]], on_update=list(si.on_update)
            )
            for w in waits[1:]:
                d2 = nc.sync.drain()
                d2.ins.sync_info = bass_rust.SyncInfo(on_wait=[w], on_update=[])
        nc.all_engine_barrier()
        assert self.sems is not None
        popped = nc._tile_sem_poison_stack.pop()
        assert popped is self._sem_poison
        nc.clear_and_free_semaphores(list(self.sems.allocated().values()))
        nc.all_engine_barrier()

    tile_mod.TileContext._drain_and_barrier = _drain_and_barrier
    tile_mod.TileContext._drain_patched = True


def _emit(tc, T):
    nc = tc.nc
    Exp = mybir.ActivationFunctionType.Exp

    from contextlib import ExitStack

    with ExitStack() as ctx:
        persist = ctx.enter_context(tc.tile_pool(name="persist", bufs=1))

        # ---- weights / persistent tiles ----
        # wk and wq live in one tile and arrive in one DMA (per-queue DMA
        # completions release at a ~2.5us-per-instruction cadence, so the
        # startup-critical path wants the fewest possible instructions)
        wkq = persist.tile([128, 8 * 2 * JC], bf, tag="wkq")
        wv = persist.tile([128, 8 * JC], bf, tag="wv")
        wo = [persist.tile([128, D], bf, tag=f"wo{i}", name=f"wo{i}") for i in range(2)]
        biasqk = persist.tile([128, 4], f32, tag="biasqk")

        # The HWDGE (sync-queue) path sustains only ~30-60 GB/s per
        # instruction; the SWDGE (gpsimd-queue) path measures ~150-200 GB/s.
        # Startup-critical transfers go on gpsimd, slack ones on sync.
        def emit_wdma(t, name, eng):
            # host pre-tiles weights as [128, 8, JC] so each partition's
            # 8*JC*2B run is contiguous (big DMA descriptors)
            eng.dma_start(
                t[:].rearrange("p (c j) -> p c j", c=8),
                T[name][:, :, :],
            )

        def emit_wodma(i):
            nc.sync.dma_start(wo[i][:], T["woT"][i * 128 : (i + 1) * 128, :])

        # per-sb q/k projection tiles ([j, s] transposed layout)
        qpS = [
            [persist.tile([128, 512], bf, tag=f"qp{j}_{s}", name=f"qp{j}_{s}")
             for s in range(NSB)]
            for j in range(2)
        ]
        kpT = [
            [persist.tile([128, 1024], bf, tag=f"kpT{i}_{th}", name=f"kpT{i}_{th}")
             for th in range(2)]
            for i in range(2)
        ]
        # per-chunk v tiles (natural [t, j] layout + ones cols)
        vpc = [persist.tile([128, VROW], bf, tag=f"vp{c}", name=f"vp{c}")
               for c in range(NC_T)]
        concatT = [persist.tile([128, S], bf, tag=f"concatT{i}", name=f"concatT{i}") for i in range(2)]

        wkq_v = wkq[:].rearrange("p (c j) -> p c j", c=8)
        wv_v = wv[:].rearrange("p (c j) -> p c j", c=8)

        q_stream = ctx.enter_context(tc.tile_pool(name="q_stream", bufs=1))
        kv_stream = ctx.enter_context(tc.tile_pool(name="kv_stream", bufs=3))
        vstream = ctx.enter_context(tc.tile_pool(name="vstream", bufs=2))
        maskp = ctx.enter_context(tc.tile_pool(name="maskp", bufs=2))
        ptp = ctx.enter_context(tc.tile_pool(name="ptp", bufs=2))
        smallp = ctx.enter_context(tc.tile_pool(name="smallp", bufs=2))
        outp = ctx.enter_context(tc.tile_pool(name="outp", bufs=1))
        scp = ctx.enter_context(tc.tile_pool(name="scp", bufs=2, space="PSUM"))
        bigp = ctx.enter_context(tc.tile_pool(name="bigp", bufs=4, space="PSUM"))
        mtiles = {}
        qtts = {}
        ktts = {}
        vtts = {}
        otiles = {}

        def emit_qdma(sb, eng=None):
            qTt = q_stream.tile([128, 8 * 512], bf, tag="qTt", name=f"qTt{sb}")
            (eng or nc.sync).dma_start(
                qTt[:].rearrange("p (c s) -> p c s", c=8),
                T["qT"][sb, :, :, :],
            )
            qtts[sb] = qTt[:].rearrange("p (c s) -> p c s", c=8)

        def emit_qproj_jt(sb, jt):
            jsl = slice(JC + jt * 128, JC + (jt + 1) * 128)
            ps = bigp.tile([128, 512], f32, tag="big", name=f"pq{sb}_{jt}")
            for c in range(8):
                nc.tensor.matmul(
                    ps[:], wkq_v[:, c, jsl], qtts[sb][:, c, :],
                    start=(c == 0), stop=(c == 7),
                )
            nc.vector.tensor_scalar_add(
                qpS[jt][sb][:], ps[:], biasqk[:, jt : jt + 1]
            )

        def emit_kdma(sb, eng=None):
            kTt = kv_stream.tile([128, 8 * 512], bf, tag="kTt", name=f"kTt{sb}")
            (eng or nc.gpsimd).dma_start(
                kTt[:].rearrange("p (c s) -> p c s", c=8),
                T["kT"][sb, :, :, :],
            )
            ktts[sb] = kTt[:].rearrange("p (c s) -> p c s", c=8)

        def emit_kproj_jt(sb, jt):
            jsl = slice(jt * 128, (jt + 1) * 128)
            ps = bigp.tile([128, 512], f32, tag="big", name=f"pk{sb}_{jt}")
            for c in range(8):
                nc.tensor.matmul(
                    ps[:], wkq_v[:, c, jsl], ktts[sb][:, c, :],
                    start=(c == 0), stop=(c == 7),
                )
            nc.vector.tensor_scalar_add(
                kpT[jt][sb // 2][:, (sb % 2) * 512 : (sb % 2 + 1) * 512],
                ps[:], biasqk[:, 2 + jt : 3 + jt]
            )

        def emit_mask_dma(sb, hf):
            mT = maskp.tile([128, 8 * 512], bf, tag="mT", name=f"mT{sb}_{hf}")
            nc.gpsimd.dma_start(
                mT[:].rearrange("p (c s) -> p c s", c=8),
                T["maskT"][sb, :, hf * 8 : (hf + 1) * 8, :],
            )
            mtiles[(sb, hf)] = mT

        def emit_vdma(tb, eng=None):
            vTt = vstream.tile([128, 8 * 512], bf, tag="vTt", name=f"vTt{tb}")
            (eng or nc.gpsimd).dma_start(
                vTt[:].rearrange("p (c s) -> p c s", c=8),
                T["vT"][tb, :, :, :],
            )
            vtts[tb] = vTt[:].rearrange("p (c t) -> p c t", c=8)

        def emit_vproj(chunk):
            tb, tt = chunk // 4, chunk % 4
            vTt_v = vtts[tb]
            ps = bigp.tile([128, 512], f32, tag="big", name=f"pv{chunk}")
            for c in range(8):
                nc.tensor.matmul(
                    ps[:, 0:JC],
                    vTt_v[:, c, tt * 128 : (tt + 1) * 128],
                    wv_v[:, c, :],
                    start=(c == 0), stop=(c == 7),
                )
            vt = vpc[chunk]
            nc.gpsimd.memset(
                vt[:].rearrange("p (h d) -> p h d", d=128)[:, :, 64:128],
                1.0,
            )
            dst = vt[:].rearrange("p (h d) -> p h d", h=HPC)[:, :, 0:DK]
            src = ps[:, 0:JC].rearrange("p (h d) -> p h d", h=HPC)
            nc.vector.tensor_copy(dst, src)

        def emit_wo_group(sb, st, mt):
            # out partial in bf16 (summed in fp32 on host); all 8 groups of
            # an s-block accumulate into one wide tile, flushed by a single
            # 1MB DMA on the fast gpsimd queue when the last group lands.
            s0 = sb * 512 + st * 128
            msl = slice(mt * 512, (mt + 1) * 512)
            pw = bigp.tile([128, 512], f32, tag="big", name=f"pw{sb}_{st}_{mt}")
            for kc in range(2):
                nc.tensor.matmul(
                    pw[:],
                    concatT[kc][:, s0 : s0 + 128],
                    wo[kc][:, msl],
                    start=(kc == 0), stop=(kc == 1),
                )
            if (st, mt) == (0, 0):
                otiles[sb] = outp.tile(
                    [128, 4096], bf, tag="ot", name=f"ot{sb}"
                )
            ot = otiles[sb]
            dst = ot[:, st * 1024 + mt * 512 : st * 1024 + (mt + 1) * 512]
            if sb == NSB - 1 and mt == 0:
                nc.scalar.copy(dst, pw[:])   # tail: ACT is idle, split load
            else:
                nc.vector.tensor_copy(dst, pw[:])
            if (st, mt) == (3, 1):
                nc.gpsimd.dma_start(
                    T["out_p"][sb * 512 : (sb + 1) * 512, :].rearrange(
                        "(t p) m -> p t m", p=128
                    ),
                    ot[:].rearrange("p (t m) -> p t m", t=4),
                )

        def emit_norm(sb, pair, po2):
            # po2 rows 0-63 hold U (unnormalized out), rows 64-127 hold the
            # denominator replicated 64x (from vpc's ones-columns), so one
            # lane-parallel reciprocal + one multiply normalizes a head.
            sl = slice(sb * 512, (sb + 1) * 512)
            for h2 in range(2):
                h = pair * 2 + h2
                psl = slice(h2 * 64, h2 * 64 + 64)
                po = po2[h2]
                rcs = smallp.tile([64, 512], f32, tag="rcs", name=f"rcs{sb}_{h}")
                nc.vector.tensor_copy(rcs[:], po[64:128, :])
                rc = smallp.tile([64, 512], f32, tag="rc", name=f"rc{sb}_{h}")
                nc.vector.reciprocal_approx_fast(rc[:], rcs[:])
                nc.vector.tensor_mul(
                    concatT[pair][psl, sl], po[0:64, :], rc[:]
                )

        # ---- static extras schedule ----
        # extras[it][c] -> list of thunks, emitted after that chunk's
        # scores+exp+attnV.  Placement is deadline-driven: a producer must be
        # EMITTED strictly before the first chunk whose instructions consume
        # it (the PE queue is in-order; a consumer emitted earlier would
        # head-of-line block on data its own queue never produces).
        extras = {it: {} for it in range(8)}

        def sched(it, c, fn):
            extras[it].setdefault(c, []).append(fn)

        # it0 (0,0): k projections (j0 feeds this iteration from chunk 4s';
        # j1 feeds it1), v projections (feed attnV during it1), q j1.
        sched(0, 1, lambda: emit_kproj_jt(1, 0))
        sched(0, 4, lambda: emit_kproj_jt(2, 0))
        sched(0, 5, lambda: emit_kproj_jt(1, 1))
        sched(0, 6, lambda: emit_kproj_jt(3, 0))
        sched(0, 7, lambda: emit_vproj(0))
        sched(0, 8, lambda: emit_kproj_jt(2, 1))
        sched(0, 9, lambda: emit_vproj(1))
        sched(0, 10, lambda: emit_kproj_jt(3, 1))
        sched(0, 11, lambda: emit_vproj(2))
        sched(0, 12, lambda: emit_qproj_jt(0, 1))
        sched(0, 13, lambda: emit_vproj(3))
        sched(0, 14, lambda: emit_vproj(4))
        sched(0, 14, lambda: emit_vdma(2))
        sched(0, 15, lambda: emit_vproj(5))
        # it1 (0,1): vproj 6..15, prefetch q(1)/mask(1)
        for i, cc in enumerate(range(1, 11)):
            sched(1, cc, lambda ch=6 + i: emit_vproj(ch))
        sched(1, 3, lambda: emit_vdma(3))
        sched(1, 8, lambda: emit_qdma(1))
        sched(1, 12, lambda: emit_qproj_jt(1, 0))
        sched(1, 14, lambda: emit_mask_dma(1, 0))
        sched(1, 15, lambda: emit_mask_dma(1, 1))
        # steady iterations
        for sb in range(1, NSB):
            it = 2 * sb
            sched(it, 0, lambda s=sb: emit_qproj_jt(s, 1))
            # wo for previous sb: its concatT is only complete after
            # norm(sb-1, 1), which runs at the END of iteration (sb, 0) —
            # so the wo groups go in iteration (sb, 1).
            for g in range(8):
                sched(it + 1, g,
                      lambda s=sb - 1, a=g // 2, b=g % 2: emit_wo_group(s, a, b))
            if sb + 1 < NSB:
                sched(it, 12, lambda s=sb + 1: emit_qdma(s))
                sched(it, 14, lambda s=sb + 1: emit_mask_dma(s, 0))
                sched(it, 15, lambda s=sb + 1: emit_mask_dma(s, 1))
                sched(it + 1, 12, lambda s=sb + 1: emit_qproj_jt(s, 0))

        # ---- prologue ----
        # The DMA engines ramp slowly from idle (~40 GB/s for the first
        # ~0.5MB), so each queue leads with a throwaway transfer.  The
        # critical path (wkq combo + k0/q0 on the scalar HWDGE ring) then
        # runs on warmed engines; wv/v0/v1/wo ride the sync queue.
        dwarm = persist.tile([128, 512], bf, tag="dwarm")
        nc.gpsimd.dma_start(
            dwarm[:, 0:256], T["qT"][0, :, 0, 0:256]
        )
        nc.sync.dma_start(
            dwarm[:, 256:512], T["qT"][0, :, 1, 0:256]
        )
        nc.gpsimd.dma_start(biasqk[:], T["biasqk"][:, :])
        emit_wdma(wkq, "wkq", nc.gpsimd)
        emit_kdma(0, nc.scalar)
        emit_qdma(0, nc.scalar)
        emit_kdma(1)
        emit_kdma(2)
        emit_wdma(wv, "wvT", nc.sync)
        emit_vdma(0, nc.sync)
        emit_vdma(1, nc.sync)
        emit_wodma(0)
        emit_wodma(1)
        # HAM warm-up: ~4us of dummy matmuls while the first DMAs land, so
        # the real projections start at the 2.4 GHz clock (K=8/8).
        warm = persist.tile([128, 512], bf, tag="warm")
        nc.gpsimd.memset(warm[:], 0.0)
        wps = bigp.tile([128, 512], f32, tag="big", name="warmps")
        for i in range(10):
            nc.tensor.matmul(
                wps[:], warm[:, 0:128], warm[:],
                start=(i == 0), stop=(i == 9),
            )
        emit_kproj_jt(0, 0)
        emit_kproj_jt(0, 1)
        emit_qproj_jt(0, 0)
        # kTt(3) recycles kTt(0)'s buffer: kp(0, *) must be emitted first
        emit_kdma(3)
        emit_mask_dma(0, 0)
        emit_mask_dma(0, 1)

        # ---- main pipeline ----
        # Per chunk: attnV(i-1) first (deps always stale -> PE never
        # head-of-line blocks), then scores(i) (waits only on the exp two
        # chunks back), then exp on ACT.  Extras fill the remaining PE slack.
        po2L = None
        prev = None        # (sb, pair, Pt)
        for sb in range(NSB):
            for pair in range(2):
                it = 2 * sb + pair
                last_it = (sb == NSB - 1 and pair == 1)

                Pt = ptp.tile(
                    [128, 2 * NC_T * 512], bf, tag="Pt", name=f"Pt{sb}_{pair}"
                )
                pv = Pt[:].rearrange("p (c h s) -> p c h s", c=NC_T, h=2)
                if prev is not None:
                    po2 = [
                        bigp.tile([128, 512], f32, tag="big",
                                  name=f"av{prev[0]}_{prev[1]}_{h2}")
                        for h2 in range(2)
                    ]
                def attn_v(dst, src_pt, c, h2, start, stop):
                    h = src_pt[1] * 2 + h2
                    nc.tensor.matmul(
                        dst[h2][:],
                        vpc[c][:, h * 128 : h * 128 + 128],
                        src_pt[2][:, (2 * c + h2) * 512 : (2 * c + h2 + 1) * 512],
                        start=start, stop=stop,
                    )

                def mask_mul(chunks):
                    hf, q = chunks[0] // 8, slice(chunks[0] % 8, chunks[0] % 8 + len(chunks))
                    mv = mtiles[(sb, hf)][:].rearrange("p (c s) -> p c s", c=8)
                    csl = slice(chunks[0], chunks[-1] + 1)
                    for h2 in range(2):
                        nc.vector.tensor_mul(
                            pv[:, csl, h2, :], pv[:, csl, h2, :], mv[:, q, :]
                        )

                cur = (sb, pair, Pt)
                for c in range(NC_T):
                    if prev is not None:
                        for h2 in range(2):
                            attn_v(po2, prev, c, h2, c == 0, c == NC_T - 1)
                    ps = scp.tile(
                        [128, 1024], f32, tag="sc", name=f"sc{sb}_{pair}_{c}"
                    )
                    for h2 in range(2):
                        psl = slice(h2 * 64, h2 * 64 + 64)
                        nc.tensor.matmul(
                            ps[:, h2 * 512 : (h2 + 1) * 512],
                            kpT[pair][c // 8][psl, (c % 8) * 128 : (c % 8 + 1) * 128],
                            qpS[pair][sb][psl, :],
                            start=True, stop=True,
                        )
                    nc.scalar.activation(
                        Pt[:, c * 1024 : (c + 1) * 1024],
                        ps[:], Exp, scale=SCALE,
                    )
                    if last_it and c >= 8:
                        # last iteration: drain our own attnV early so the
                        # tail after the final exp is as short as possible
                        if c == 8:
                            po2L = [
                                bigp.tile([128, 512], f32, tag="big",
                                          name=f"avL_{h2}")
                                for h2 in range(2)
                            ]
                        for h2 in range(2):
                            attn_v(po2L, cur, c - 8, h2, c == 8, False)
                        if c >= 12:
                            for h2 in range(2):
                                attn_v(po2L, cur, c - 4, h2, False, False)
                    if last_it:
                        if c in (7, 11, 13, 15):
                            mask_mul({7: list(range(0, 8)), 11: [8, 9, 10, 11],
                                      13: [12, 13], 15: [14, 15]}[c])
                    elif c == 7 or c == NC_T - 1:
                        mask_mul(list(range(0, 8)) if c == 7 else
                                 list(range(8, NC_T)))
                    for fn in extras[it].get(c, ()):
                        fn()
                if prev is not None:
                    emit_norm(prev[0], prev[1], po2)
                prev = cur
        # tail: finish attnv(3,1) chunks 12..15, then norm + final Wo
        psb, ppair, pPt = prev
        for c in range(12, NC_T):
            for h2 in range(2):
                attn_v(po2L, prev, c, h2, False, c == NC_T - 1)
        emit_norm(psb, ppair, po2L)
        for st in range(4):
            for mt in range(2):
                emit_wo_group(NSB - 1, st, mt)


def build_nc():
    nc = bacc.Bacc("TRN2", target_bir_lowering=False, debug=False)
    names = {}
    def din(name, shape, dt):
        names[name] = nc.dram_tensor(name, shape, dt, kind="ExternalInput").ap()
    # q/k/v pre-tiled on host to [sb, p, c, s] and mask to [sb, p, c, s] so
    # every DMA descriptor covers a full 8-16KB partition line (the
    # descriptor-generation rate, ~12ns/descriptor, caps DMA throughput
    # otherwise).
    din("qT", [NSB, 128, 8, 512], bf)
    din("kT", [NSB, 128, 8, 512], bf)
    din("vT", [NSB, 128, 8, 512], bf)
    din("maskT", [NSB, 128, NC_T, 512], bf)
    din("wkq", [128, 8, 2 * JC], bf)
    din("wvT", [128, 8, JC], bf)
    din("woT", [JC, D], bf)
    din("biasqk", [128, 4], f32)
    names["out_p"] = nc.dram_tensor(
        "out_p", [S, D], bf, kind="ExternalOutput"
    ).ap()
    with tile_mod.TileContext(nc) as tc:
        _emit(tc, names)
    nc.compile()
    return nc


_NC = None


def _tile_ds(xT, nc_):
    """[D, S] -> [NSB, 128, nc_, S // nc_ // ...] host pre-tiling.

    Element (sb, p, c, s) = xT[c * 128 + p, sb * blk + s] where blk = S/NSB.
    """
    d, s_ = xT.shape
    blk = s_ // NSB
    nch = d // 128
    # xT[(c p), (sb s)] -> [c, p, sb, s] -> [sb, p, c, s]
    r = xT.reshape(nch, 128, NSB, blk).transpose(2, 1, 0, 3)
    return np.ascontiguousarray(r)


def prep_inputs(q, k, v, mask, Wq, bq, Wk, bk, Wv, bv, Wo, bo):
    q = np.asarray(q, F32)
    k = np.asarray(k, F32)
    v = np.asarray(v, F32)
    mask = np.asarray(mask)
    Wq, Wk, Wv, Wo = (np.asarray(w, F32) for w in (Wq, Wk, Wv, Wo))
    bq, bk, bv, bo = (np.asarray(b_, F32) for b_ in (bq, bk, bv, bo))

    maskT = _tile_ds(np.ascontiguousarray(mask[0, 0].T).astype(BF16), NC_T)
    qT = [_tile_ds(q[b_].T.astype(BF16), 8) for b_ in range(B)]
    kT = [_tile_ds(k[b_].T.astype(BF16), 8) for b_ in range(B)]
    vT = [_tile_ds(v[b_].T.astype(BF16), 8) for b_ in range(B)]

    def _tile_w(wT):
        # [D, JC] -> [128, 8, JC]
        return np.ascontiguousarray(
            wT.reshape(8, 128, JC).transpose(1, 0, 2)
        )

    in_maps = []
    for c in range(N_CORES):
        b_, g = c // 4, c % 4
        js = slice(g * JC, (g + 1) * JC)
        biasqk = np.stack(
            [bq[js][:128], bq[js][128:], bk[js][:128], bk[js][128:]], axis=1
        ).astype(F32)
        in_maps.append(
            {
                "qT": qT[b_],
                "kT": kT[b_],
                "vT": vT[b_],
                "maskT": maskT,
                "wkq": np.ascontiguousarray(np.concatenate(
                    [_tile_w(Wk[js, :].T.astype(BF16)),
                     _tile_w(Wq[js, :].T.astype(BF16))], axis=2)),
                "wvT": _tile_w(Wv[js, :].T.astype(BF16)),
                "woT": np.ascontiguousarray(Wo[:, js].T).astype(BF16),
                "biasqk": np.ascontiguousarray(biasqk),
            }
        )
    # bv contributes a constant (softmax rows sum to 1): out += Wo @ bv + bo
    bias_out = (Wo @ bv + bo).astype(F32)
    return in_maps, bias_out


def run_prepped(in_maps, bias_out, trace=False, **kw):
    global _NC
    if _NC is None:
        _NC = build_nc()
    res = run_bass_kernel_spmd(
        _NC, in_maps, list(range(N_CORES)), trace=trace, **kw
    )
    out = np.zeros((B, S, D), F32)
    for c in range(N_CORES):
        out[c // 4] += res.results[c]["out_p"].astype(F32)
    out += bias_out[None, None, :]
    return out, res


def kernel(q, k, v, mask, Wq, bq, Wk, bk, Wv, bv, Wo, bo):
    in_maps, bias_out = prep_inputs(
        q, k, v, mask, Wq, bq, Wk, bk, Wv, bv, Wo, bo
    )
    out, _ = run_prepped(in_maps, bias_out)
    return out



# revision 49
# speedup vs baseline: 1.2058x; 1.0283x over previous
"""Trainium2 Bass kernel for nn_MultiHeadAttention (B=2, S=2048, D=1024, H=16).

Sharding: 8 cores = 2 (batch) x 4 (head groups of 4 heads / 256 dims).
Each core computes QKV projections for its head slice, attention for its 4
heads, and the partial output projection for its 256-dim slice of Wo's input.
Host sums the 4 partials per batch element (Megatron-style row-parallel Wo).

Device layouts (per core):
  qT/kT/vT  [1024, 2048] bf16   (input, transposed on host)
  wqT/wkT/wvT [1024, 256] bf16  (Wq[js].T etc)
  woT       [256, 1024] bf16    (Wo[:, js].T)
  maskT     [2048, 2048] bf16   (mask[0,0].T as 0.0/1.0)
  qpT/kpT   [256(j), 2048(s)]   (projections, transposed: j on partitions)
  vp        [2048(t), 4x65]     (natural layout; col 64 of each 65-block = 1.0
                                 -> attn@V matmul also produces softmax denom)
  P~        [t, s] = exp(scoresT/8) * maskT   (scoresT = K_h.T^T @ Q_h.T)
  attn out  [65(j+denom), s] -> normalized -> concatT [256(j), 2048(s)]
  out_p     [2048, 1024] f32 partial = concatT.T @ woT
"""

import sys

import numpy as np

try:
    import concourse.bass as bass
except ImportError:  # pragma: no cover
    sys.path.insert(0, "/opt/trn_rl_repo")
    import concourse.bass as bass

from concourse import bacc

import ml_dtypes

import concourse.tile as tile_mod
from concourse import mybir
from concourse.bass_utils import run_bass_kernel_spmd

BF16 = ml_dtypes.bfloat16
F32 = np.float32

B, S, D, H = 2, 2048, 1024, 16
DK = D // H            # 64
N_CORES = 8
HPC = 4                # heads per core
JC = HPC * DK          # 256 j-dims per core
SCALE = 1.0 / float(np.sqrt(DK))
NSB = S // 512         # 4 s-blocks
NC_T = S // 128        # 16 t-chunks
VROW = HPC * 128       # 512: [h0 64dims | 64 ones | h1 ...]; the 64
                       # ones-columns make attnV emit the softmax denom
                       # replicated on 64 psum partitions (free: matmul
                       # time is column-count of the moving operand)

bf = mybir.dt.bfloat16
f32 = mybir.dt.float32


def _patch_drain():
    """This walrus build only accepts 1 sync-wait per instruction; the Tile
    exit drain carries one wait per pending proc. Split them across drains."""
    if getattr(tile_mod.TileContext, "_drain_patched", False):
        return
    import bass_rust

    def _drain_and_barrier(self, tick_clock, wait_clock):
        from concourse.tile import ScopedClock

        nc = self.nc
        drain_inst = nc.sync.drain()
        wait_clock.add_sem_waits(
            drain_inst.ins, ScopedClock({None: tick_clock.global_clock})
        )
        si = drain_inst.ins.sync_info
        waits = list(si.on_wait)
        if len(waits) > 1:
            drain_inst.ins.sync_info = bass_rust.SyncInfo(
                on_wait=[waits[0]], on_update=list(si.on_update)
            )
            for w in waits[1:]:
                d2 = nc.sync.drain()
                d2.ins.sync_info = bass_rust.SyncInfo(on_wait=[w], on_update=[])
        nc.all_engine_barrier()
        assert self.sems is not None
        popped = nc._tile_sem_poison_stack.pop()
        assert popped is self._sem_poison
        nc.clear_and_free_semaphores(list(self.sems.allocated().values()))
        nc.all_engine_barrier()

    tile_mod.TileContext._drain_and_barrier = _drain_and_barrier
    tile_mod.TileContext._drain_patched = True


def _emit(tc, T):
    nc = tc.nc
    Exp = mybir.ActivationFunctionType.Exp

    from contextlib import ExitStack

    with ExitStack() as ctx:
        persist = ctx.enter_context(tc.tile_pool(name="persist", bufs=1))

        # ---- weights / persistent tiles ----
        # wk and wq live in one tile and arrive in one DMA (per-queue DMA
        # completions release at a ~2.5us-per-instruction cadence, so the
        # startup-critical path wants the fewest possible instructions)
        wkq = persist.tile([128, 8 * 2 * JC], bf, tag="wkq")
        wv = persist.tile([128, 8 * JC], bf, tag="wv")
        wo = [persist.tile([128, D], bf, tag=f"wo{i}", name=f"wo{i}") for i in range(2)]
        biasqk = persist.tile([128, 4], f32, tag="biasqk")

        # The HWDGE (sync-queue) path sustains only ~30-60 GB/s per
        # instruction; the SWDGE (gpsimd-queue) path measures ~150-200 GB/s.
        # Startup-critical transfers go on gpsimd, slack ones on sync.
        def emit_wdma(t, name, eng):
            # host pre-tiles weights as [128, 8, JC] so each partition's
            # 8*JC*2B run is contiguous (big DMA descriptors)
            eng.dma_start(
                t[:].rearrange("p (c j) -> p c j", c=8),
                T[name][:, :, :],
            )

        def emit_wodma(i):
            nc.sync.dma_start(wo[i][:], T["woT"][i * 128 : (i + 1) * 128, :])

        # per-sb q/k projection tiles ([j, s] transposed layout)
        qpS = [
            [persist.tile([128, 512], bf, tag=f"qp{j}_{s}", name=f"qp{j}_{s}")
             for s in range(NSB)]
            for j in range(2)
        ]
        kpT = [
            [persist.tile([128, 1024], bf, tag=f"kpT{i}_{th}", name=f"kpT{i}_{th}")
             for th in range(2)]
            for i in range(2)
        ]
        # per-chunk v tiles (natural [t, j] layout + ones cols)
        vpc = [persist.tile([128, VROW], bf, tag=f"vp{c}", name=f"vp{c}")
               for c in range(NC_T)]
        concatT = [persist.tile([128, S], bf, tag=f"concatT{i}", name=f"concatT{i}") for i in range(2)]

        wkq_v = wkq[:].rearrange("p (c j) -> p c j", c=8)
        wv_v = wv[:].rearrange("p (c j) -> p c j", c=8)

        q_stream = ctx.enter_context(tc.tile_pool(name="q_stream", bufs=1))
        kv_stream = ctx.enter_context(tc.tile_pool(name="kv_stream", bufs=3))
        vstream = ctx.enter_context(tc.tile_pool(name="vstream", bufs=2))
        maskp = ctx.enter_context(tc.tile_pool(name="maskp", bufs=2))
        ptp = ctx.enter_context(tc.tile_pool(name="ptp", bufs=2))
        smallp = ctx.enter_context(tc.tile_pool(name="smallp", bufs=2))
        outp = ctx.enter_context(tc.tile_pool(name="outp", bufs=1))
        scp = ctx.enter_context(tc.tile_pool(name="scp", bufs=2, space="PSUM"))
        bigp = ctx.enter_context(tc.tile_pool(name="bigp", bufs=4, space="PSUM"))
        mtiles = {}
        qtts = {}
        ktts = {}
        vtts = {}
        otiles = {}

        def emit_qdma(sb, eng=None):
            qTt = q_stream.tile([128, 8 * 512], bf, tag="qTt", name=f"qTt{sb}")
            (eng or nc.sync).dma_start(
                qTt[:].rearrange("p (c s) -> p c s", c=8),
                T["qT"][sb, :, :, :],
            )
            qtts[sb] = qTt[:].rearrange("p (c s) -> p c s", c=8)

        def emit_qproj_jt(sb, jt):
            jsl = slice(JC + jt * 128, JC + (jt + 1) * 128)
            ps = bigp.tile([128, 512], f32, tag="big", name=f"pq{sb}_{jt}")
            for c in range(8):
                nc.tensor.matmul(
                    ps[:], wkq_v[:, c, jsl], qtts[sb][:, c, :],
                    start=(c == 0), stop=(c == 7),
                )
            nc.vector.tensor_scalar_add(
                qpS[jt][sb][:], ps[:], biasqk[:, jt : jt + 1]
            )

        def emit_kdma(sb, eng=None):
            kTt = kv_stream.tile([128, 8 * 512], bf, tag="kTt", name=f"kTt{sb}")
            (eng or nc.gpsimd).dma_start(
                kTt[:].rearrange("p (c s) -> p c s", c=8),
                T["kT"][sb, :, :, :],
            )
            ktts[sb] = kTt[:].rearrange("p (c s) -> p c s", c=8)

        def emit_kproj_jt(sb, jt):
            jsl = slice(jt * 128, (jt + 1) * 128)
            ps = bigp.tile([128, 512], f32, tag="big", name=f"pk{sb}_{jt}")
            for c in range(8):
                nc.tensor.matmul(
                    ps[:], wkq_v[:, c, jsl], ktts[sb][:, c, :],
                    start=(c == 0), stop=(c == 7),
                )
            nc.vector.tensor_scalar_add(
                kpT[jt][sb // 2][:, (sb % 2) * 512 : (sb % 2 + 1) * 512],
                ps[:], biasqk[:, 2 + jt : 3 + jt]
            )

        def emit_mask_dma(sb, hf):
            mT = maskp.tile([128, 8 * 512], bf, tag="mT", name=f"mT{sb}_{hf}")
            nc.gpsimd.dma_start(
                mT[:].rearrange("p (c s) -> p c s", c=8),
                T["maskT"][sb, :, hf * 8 : (hf + 1) * 8, :],
            )
            mtiles[(sb, hf)] = mT

        def emit_vdma(tb, eng=None):
            vTt = vstream.tile([128, 8 * 512], bf, tag="vTt", name=f"vTt{tb}")
            (eng or nc.gpsimd).dma_start(
                vTt[:].rearrange("p (c s) -> p c s", c=8),
                T["vT"][tb, :, :, :],
            )
            vtts[tb] = vTt[:].rearrange("p (c t) -> p c t", c=8)

        def emit_vproj(chunk):
            tb, tt = chunk // 4, chunk % 4
            vTt_v = vtts[tb]
            ps = bigp.tile([128, 512], f32, tag="big", name=f"pv{chunk}")
            for c in range(8):
                nc.tensor.matmul(
                    ps[:, 0:JC],
                    vTt_v[:, c, tt * 128 : (tt + 1) * 128],
                    wv_v[:, c, :],
                    start=(c == 0), stop=(c == 7),
                )
            vt = vpc[chunk]
            nc.gpsimd.memset(
                vt[:].rearrange("p (h d) -> p h d", d=128)[:, :, 64:128],
                1.0,
            )
            dst = vt[:].rearrange("p (h d) -> p h d", h=HPC)[:, :, 0:DK]
            src = ps[:, 0:JC].rearrange("p (h d) -> p h d", h=HPC)
            nc.vector.tensor_copy(dst, src)

        def emit_wo_group(sb, st, mt):
            # out partial in bf16 (summed in fp32 on host); all 8 groups of
            # an s-block accumulate into one wide tile, flushed by a single
            # 1MB DMA on the fast gpsimd queue when the last group lands.
            s0 = sb * 512 + st * 128
            msl = slice(mt * 512, (mt + 1) * 512)
            pw = bigp.tile([128, 512], f32, tag="big", name=f"pw{sb}_{st}_{mt}")
            for kc in range(2):
                nc.tensor.matmul(
                    pw[:],
                    concatT[kc][:, s0 : s0 + 128],
                    wo[kc][:, msl],
                    start=(kc == 0), stop=(kc == 1),
                )
            if (st, mt) == (0, 0):
                otiles[sb] = outp.tile(
                    [128, 4096], bf, tag="ot", name=f"ot{sb}"
                )
            ot = otiles[sb]
            dst = ot[:, st * 1024 + mt * 512 : st * 1024 + (mt + 1) * 512]
            if sb == NSB - 1 and mt == 0:
                nc.scalar.copy(dst, pw[:])   # tail: ACT is idle, split load
            else:
                nc.vector.tensor_copy(dst, pw[:])
            if (st, mt) == (3, 1):
                nc.gpsimd.dma_start(
                    T["out_p"][sb * 512 : (sb + 1) * 512, :].rearrange(
                        "(t p) m -> p t m", p=128
                    ),
                    ot[:].rearrange("p (t m) -> p t m", t=4),
                )

        def emit_norm(sb, pair, po2):
            # po2 rows 0-63 hold U (unnormalized out), rows 64-127 hold the
            # denominator replicated 64x (from vpc's ones-columns), so one
            # lane-parallel reciprocal + one multiply normalizes a head.
            sl = slice(sb * 512, (sb + 1) * 512)
            for h2 in range(2):
                h = pair * 2 + h2
                psl = slice(h2 * 64, h2 * 64 + 64)
                po = po2[h2]
                rcs = smallp.tile([64, 512], f32, tag="rcs", name=f"rcs{sb}_{h}")
                nc.vector.tensor_copy(rcs[:], po[64:128, :])
                rc = smallp.tile([64, 512], f32, tag="rc", name=f"rc{sb}_{h}")
                nc.vector.reciprocal_approx_fast(rc[:], rcs[:])
                nc.vector.tensor_mul(
                    concatT[pair][psl, sl], po[0:64, :], rc[:]
                )

        # ---- static extras schedule ----
        # extras[it][c] -> list of thunks, emitted after that chunk's
        # scores+exp+attnV.  Placement is deadline-driven: a producer must be
        # EMITTED strictly before the first chunk whose instructions consume
        # it (the PE queue is in-order; a consumer emitted earlier would
        # head-of-line block on data its own queue never produces).
        extras = {it: {} for it in range(8)}

        def sched(it, c, fn):
            extras[it].setdefault(c, []).append(fn)

        # it0 (0,0): k projections (j0 feeds this iteration from chunk 4s';
        # j1 feeds it1), v projections (feed attnV during it1), q j1.
        sched(0, 1, lambda: emit_kproj_jt(1, 0))
        sched(0, 3, lambda: emit_kproj_jt(1, 1))
        sched(0, 5, lambda: emit_kproj_jt(2, 0))
        sched(0, 6, lambda: emit_vproj(0))
        sched(0, 7, lambda: emit_kproj_jt(2, 1))
        sched(0, 8, lambda: emit_kproj_jt(3, 0))
        sched(0, 9, lambda: emit_vproj(1))
        sched(0, 10, lambda: emit_vproj(2))
        sched(0, 11, lambda: emit_qproj_jt(0, 1))
        sched(0, 12, lambda: emit_kproj_jt(3, 1))
        sched(0, 13, lambda: emit_vproj(3))
        sched(0, 14, lambda: emit_vproj(4))
        sched(0, 14, lambda: emit_vdma(2))
        sched(0, 15, lambda: emit_vproj(5))
        # it1 (0,1): vproj 6..15, prefetch q(1)/mask(1)
        for i, cc in enumerate(range(1, 11)):
            sched(1, cc, lambda ch=6 + i: emit_vproj(ch))
        sched(1, 3, lambda: emit_vdma(3))
        sched(1, 8, lambda: emit_qdma(1))
        sched(1, 12, lambda: emit_qproj_jt(1, 0))
        sched(1, 14, lambda: emit_mask_dma(1, 0))
        sched(1, 15, lambda: emit_mask_dma(1, 1))
        # steady iterations
        for sb in range(1, NSB):
            it = 2 * sb
            sched(it, 0, lambda s=sb: emit_qproj_jt(s, 1))
            # wo for previous sb: its concatT is only complete after
            # norm(sb-1, 1), which runs at the END of iteration (sb, 0) —
            # so the wo groups go in iteration (sb, 1).
            for g in range(8):
                sched(it + 1, g,
                      lambda s=sb - 1, a=g // 2, b=g % 2: emit_wo_group(s, a, b))
            if sb + 1 < NSB:
                sched(it, 12, lambda s=sb + 1: emit_qdma(s))
                sched(it, 14, lambda s=sb + 1: emit_mask_dma(s, 0))
                sched(it, 15, lambda s=sb + 1: emit_mask_dma(s, 1))
                sched(it + 1, 12, lambda s=sb + 1: emit_qproj_jt(s, 0))

        # ---- prologue ----
        # The DMA engines ramp slowly from idle (~40 GB/s for the first
        # ~0.5MB), so each queue leads with a throwaway transfer.  The
        # critical path (wkq combo + k0/q0 on the scalar HWDGE ring) then
        # runs on warmed engines; wv/v0/v1/wo ride the sync queue.
        dwarm = persist.tile([128, 512], bf, tag="dwarm")
        nc.gpsimd.dma_start(
            dwarm[:, 0:256], T["qT"][0, :, 0, 0:256]
        )
        nc.sync.dma_start(
            dwarm[:, 256:512], T["qT"][0, :, 1, 0:256]
        )
        nc.sync.dma_start(biasqk[:], T["biasqk"][:, :])
        emit_kdma(0, nc.sync)
        emit_wdma(wkq, "wkq", nc.gpsimd)
        emit_qdma(0, nc.gpsimd)
        emit_kdma(1)
        emit_kdma(2)
        emit_wdma(wv, "wvT", nc.sync)
        emit_vdma(0, nc.sync)
        emit_vdma(1, nc.sync)
        emit_wodma(0)
        emit_wodma(1)
        # HAM warm-up: dummy matmuls bridge the PE from t~6.5us until the
        # first projection inputs land (~20us), so nothing runs at the cold
        # 1.2 GHz clock.  memset on DVE: the gpsimd queue is busy with
        # DMA descriptor generation at t=0.
        warm = persist.tile([128, 512], bf, tag="warm")
        nc.vector.memset(warm[:], 0.0)
        wps = bigp.tile([128, 512], f32, tag="big", name="warmps")
        for i in range(36):
            nc.tensor.matmul(
                wps[:], warm[:, 0:128], warm[:],
                start=(i == 0), stop=(i == 35),
            )
        emit_kproj_jt(0, 0)
        emit_kproj_jt(0, 1)
        emit_qproj_jt(0, 0)
        emit_mask_dma(0, 0)
        emit_mask_dma(0, 1)
        # kTt(3) recycles kTt(0)'s buffer, so its DMA instruction carries a
        # sem-wait on kp(0,*) that would head-of-line block the gpsimd DMA
        # ring — it goes last, after the masks.
        emit_kdma(3)

        # ---- main pipeline ----
        # Per chunk: attnV(i-1) first (deps always stale -> PE never
        # head-of-line blocks), then scores(i) (waits only on the exp two
        # chunks back), then exp on ACT.  Extras fill the remaining PE slack.
        po2L = None
        prev = None        # (sb, pair, Pt)
        for sb in range(NSB):
            for pair in range(2):
                it = 2 * sb + pair
                last_it = (sb == NSB - 1 and pair == 1)

                Pt = ptp.tile(
                    [128, 2 * NC_T * 512], bf, tag="Pt", name=f"Pt{sb}_{pair}"
                )
                pv = Pt[:].rearrange("p (c h s) -> p c h s", c=NC_T, h=2)
                if prev is not None:
                    po2 = [
                        bigp.tile([128, 512], f32, tag="big",
                                  name=f"av{prev[0]}_{prev[1]}_{h2}")
                        for h2 in range(2)
                    ]
                def attn_v(dst, src_pt, c, h2, start, stop):
                    h = src_pt[1] * 2 + h2
                    nc.tensor.matmul(
                        dst[h2][:],
                        vpc[c][:, h * 128 : h * 128 + 128],
                        src_pt[2][:, (2 * c + h2) * 512 : (2 * c + h2 + 1) * 512],
                        start=start, stop=stop,
                    )

                def mask_mul(chunks):
                    hf, q = chunks[0] // 8, slice(chunks[0] % 8, chunks[0] % 8 + len(chunks))
                    mv = mtiles[(sb, hf)][:].rearrange("p (c s) -> p c s", c=8)
                    csl = slice(chunks[0], chunks[-1] + 1)
                    for h2 in range(2):
                        nc.vector.tensor_mul(
                            pv[:, csl, h2, :], pv[:, csl, h2, :], mv[:, q, :]
                        )

                cur = (sb, pair, Pt)
                for c in range(NC_T):
                    if prev is not None:
                        for h2 in range(2):
                            attn_v(po2, prev, c, h2, c == 0, c == NC_T - 1)
                    ps = scp.tile(
                        [128, 1024], f32, tag="sc", name=f"sc{sb}_{pair}_{c}"
                    )
                    for h2 in range(2):
                        psl = slice(h2 * 64, h2 * 64 + 64)
                        nc.tensor.matmul(
                            ps[:, h2 * 512 : (h2 + 1) * 512],
                            kpT[pair][c // 8][psl, (c % 8) * 128 : (c % 8 + 1) * 128],
                            qpS[pair][sb][psl, :],
                            start=True, stop=True,
                        )
                    nc.scalar.activation(
                        Pt[:, c * 1024 : (c + 1) * 1024],
                        ps[:], Exp, scale=SCALE,
                    )
                    if last_it and c >= 8:
                        # last iteration: drain our own attnV early so the
                        # tail after the final exp is as short as possible
                        if c == 8:
                            po2L = [
                                bigp.tile([128, 512], f32, tag="big",
                                          name=f"avL_{h2}")
                                for h2 in range(2)
                            ]
                        for h2 in range(2):
                            attn_v(po2L, cur, c - 8, h2, c == 8, False)
                        if c >= 12:
                            for h2 in range(2):
                                attn_v(po2L, cur, c - 4, h2, False, False)
                    if last_it:
                        if c in (7, 11, 13, 15):
                            mask_mul({7: list(range(0, 8)), 11: [8, 9, 10, 11],
                                      13: [12, 13], 15: [14, 15]}[c])
                    elif c == 7 or c == NC_T - 1:
                        mask_mul(list(range(0, 8)) if c == 7 else
                                 list(range(8, NC_T)))
                    for fn in extras[it].get(c, ()):
                        fn()
                if prev is not None:
                    emit_norm(prev[0], prev[1], po2)
                prev = cur
        # tail: finish attnv(3,1) chunks 12..15, then norm + final Wo
        psb, ppair, pPt = prev
        for c in range(12, NC_T):
            for h2 in range(2):
                attn_v(po2L, prev, c, h2, False, c == NC_T - 1)
        emit_norm(psb, ppair, po2L)
        for st in range(4):
            for mt in range(2):
                emit_wo_group(NSB - 1, st, mt)


def build_nc():
    nc = bacc.Bacc("TRN2", target_bir_lowering=False, debug=False)
    names = {}
    def din(name, shape, dt):
        names[name] = nc.dram_tensor(name, shape, dt, kind="ExternalInput").ap()
    # q/k/v pre-tiled on host to [sb, p, c, s] and mask to [sb, p, c, s] so
    # every DMA descriptor covers a full 8-16KB partition line (the
    # descriptor-generation rate, ~12ns/descriptor, caps DMA throughput
    # otherwise).
    din("qT", [NSB, 128, 8, 512], bf)
    din("kT", [NSB, 128, 8, 512], bf)
    din("vT", [NSB, 128, 8, 512], bf)
    din("maskT", [NSB, 128, NC_T, 512], bf)
    din("wkq", [128, 8, 2 * JC], bf)
    din("wvT", [128, 8, JC], bf)
    din("woT", [JC, D], bf)
    din("biasqk", [128, 4], f32)
    names["out_p"] = nc.dram_tensor(
        "out_p", [S, D], bf, kind="ExternalOutput"
    ).ap()
    with tile_mod.TileContext(nc) as tc:
        _emit(tc, names)
    nc.compile()
    return nc


_NC = None


def _tile_ds(xT, nc_):
    """[D, S] -> [NSB, 128, nc_, S // nc_ // ...] host pre-tiling.

    Element (sb, p, c, s) = xT[c * 128 + p, sb * blk + s] where blk = S/NSB.
    """
    d, s_ = xT.shape
    blk = s_ // NSB
    nch = d // 128
    # xT[(c p), (sb s)] -> [c, p, sb, s] -> [sb, p, c, s]
    r = xT.reshape(nch, 128, NSB, blk).transpose(2, 1, 0, 3)
    return np.ascontiguousarray(r)


def prep_inputs(q, k, v, mask, Wq, bq, Wk, bk, Wv, bv, Wo, bo):
    q = np.asarray(q, F32)
    k = np.asarray(k, F32)
    v = np.asarray(v, F32)
    mask = np.asarray(mask)
    Wq, Wk, Wv, Wo = (np.asarray(w, F32) for w in (Wq, Wk, Wv, Wo))
    bq, bk, bv, bo = (np.asarray(b_, F32) for b_ in (bq, bk, bv, bo))

    maskT = _tile_ds(np.ascontiguousarray(mask[0, 0].T).astype(BF16), NC_T)
    qT = [_tile_ds(q[b_].T.astype(BF16), 8) for b_ in range(B)]
    kT = [_tile_ds(k[b_].T.astype(BF16), 8) for b_ in range(B)]
    vT = [_tile_ds(v[b_].T.astype(BF16), 8) for b_ in range(B)]

    def _tile_w(wT):
        # [D, JC] -> [128, 8, JC]
        return np.ascontiguousarray(
            wT.reshape(8, 128, JC).transpose(1, 0, 2)
        )

    in_maps = []
    for c in range(N_CORES):
        b_, g = c // 4, c % 4
        js = slice(g * JC, (g + 1) * JC)
        biasqk = np.stack(
            [bq[js][:128], bq[js][128:], bk[js][:128], bk[js][128:]], axis=1
        ).astype(F32)
        in_maps.append(
            {
                "qT": qT[b_],
                "kT": kT[b_],
                "vT": vT[b_],
                "maskT": maskT,
                "wkq": np.ascontiguousarray(np.concatenate(
                    [_tile_w(Wk[js, :].T.astype(BF16)),
                     _tile_w(Wq[js, :].T.astype(BF16))], axis=2)),
                "wvT": _tile_w(Wv[js, :].T.astype(BF16)),
                "woT": np.ascontiguousarray(Wo[:, js].T).astype(BF16),
                "biasqk": np.ascontiguousarray(biasqk),
            }
        )
    # bv contributes a constant (softmax rows sum to 1): out += Wo @ bv + bo
    bias_out = (Wo @ bv + bo).astype(F32)
    return in_maps, bias_out


def run_prepped(in_maps, bias_out, trace=False, **kw):
    global _NC
    if _NC is None:
        _NC = build_nc()
    res = run_bass_kernel_spmd(
        _NC, in_maps, list(range(N_CORES)), trace=trace, **kw
    )
    out = np.zeros((B, S, D), F32)
    for c in range(N_CORES):
        out[c // 4] += res.results[c]["out_p"].astype(F32)
    out += bias_out[None, None, :]
    return out, res


def kernel(q, k, v, mask, Wq, bq, Wk, bk, Wv, bv, Wo, bo):
    in_maps, bias_out = prep_inputs(
        q, k, v, mask, Wq, bq, Wk, bk, Wv, bv, Wo, bo
    )
    out, _ = run_prepped(in_maps, bias_out)
    return out

